# revision 11
# baseline (speedup 1.0000x reference)
"""Trainium2 Bass kernel for AngularPenaltySMLossWithSoftLabel.

Strategy: tensor-parallel over out_features C=10000 across 8 cores
(1250 columns each). Each core:
  - normalizes x rows (full batch), transposes xn and its W shard on PE,
  - computes its wf column shard  wf[:, c_lo:c_hi] = xn @ W_shard.T,
  - accumulates per-row sum_j exp(S*wf) over its shard (fused in ACT exp),
  - gathers a 192-wide window around each row's label (clipped to its
    shard) from its wf shard via indirect DMA and computes the soft-label
    kernel * arcface-numerator contribution (exact windowed sum; the
    exp(-0.2*d) kernel is < 3e-6 outside +-64),
  - AllReduces [exp_sum, term1, wf_y] (3 x 2048 f32) across cores,
  - computes the final per-row loss vector L on device.
Host glue: concatenate wf shards, slice padding, loss = -mean(L).
"""

import math
import numpy as np

N, D, C = 2048, 512, 10000
NCORES = 8
CSH = C // NCORES          # 1250 columns per core
CPAD = 1280                # padded row stride of the wf output (256B multiple)
S_SCALE = 64.0
MARGIN = 0.5
EPS = 1e-7
KDECAY = 0.2

A_COS = S_SCALE * math.cos(MARGIN)            # 56.1652...
B_SIN = -S_SCALE * math.sin(MARGIN)           # -30.6832...
A_OVER_B = math.cos(MARGIN) / -math.sin(MARGIN)  # -1.8304877...
CLIP = 1.0 - EPS
G = math.exp(-KDECAY)
C2 = 1.0 / (1.0 - G)                          # 5.51667
C1 = 1.0 + 2.0 * G * C2                       # 10.03331

WIN = 192          # gather window width (64-aligned, covers label +-64)
BLK = 64           # gather stride granularity (256B in f32)
BMAX = (CPAD - WIN) // BLK   # 17: max window block start
NG = N // 128      # 16 row-chunks of 128
CW = [512, 512, 256]         # c-chunk widths (local columns)
CVALID = [512, 512, CSH - 1024]  # valid (non-ghost) widths

_CACHE = {}


def _build():
    import concourse.bass as bass
    import concourse.mybir as mybir
    from concourse import bacc, tile
    from concourse.masks import make_identity
    from contextlib import ExitStack

    f32 = mybir.dt.float32
    i32 = mybir.dt.int32
    i16 = mybir.dt.int16
    Alu = mybir.AluOpType
    Act = mybir.ActivationFunctionType

    nc = bacc.Bacc("TRN2", num_devices=NCORES)

    x_ext = nc.declare_dram_parameter("x", [N, D], f32, isOutput=False)
    w_ext = nc.declare_dram_parameter("w", [CSH, D], f32, isOutput=False)
    lab_ext = nc.declare_dram_parameter("labels", [N], i32, isOutput=False)
    ncl_ext = nc.declare_dram_parameter("negclo", [128, 1], i32, isOutput=False)
    wf_ext = nc.declare_dram_parameter("wf_out", [N, CPAD], f32, isOutput=True)
    l_ext = nc.declare_dram_parameter("l_out", [N], f32, isOutput=True)

    ar_in = nc.dram_tensor("ar_in", [128 * 48], f32)
    ar_out = nc.dram_tensor("ar_out", [128 * 48], f32, addr_space="Shared")

    with ExitStack() as ctx:
        tc = ctx.enter_context(tile.TileContext(nc))
        singles = ctx.enter_context(tc.tile_pool(name="singles", bufs=1))
        xt_pool = ctx.enter_context(tc.tile_pool(name="xt", bufs=3))
        wt_pool = ctx.enter_context(tc.tile_pool(name="wt", bufs=3))
        ps_t = ctx.enter_context(tc.tile_pool(name="pst", bufs=4, space="PSUM"))
        ps_mm = ctx.enter_context(tc.tile_pool(name="psmm", bufs=3, space="PSUM"))
        wf_pool = ctx.enter_context(tc.tile_pool(name="wfout", bufs=4))
        dump_pool = ctx.enter_context(tc.tile_pool(name="edump", bufs=2))
        work = ctx.enter_context(tc.tile_pool(name="work", bufs=2))

        def bcast(ap, reps, width):
            """[128, k] slice -> [128, k, width] with stride-0 inner dim."""
            assert len(ap.ap) == 2
            return bass.AP(
                tensor=ap.tensor,
                offset=ap.offset,
                ap=[list(ap.ap[0]), list(ap.ap[1]), [0, width]],
            )

        def bcast_col(ap, width):
            """[P, 1] column -> [P, width] with stride-0 free dim."""
            return bass.AP(
                tensor=ap.tensor,
                offset=ap.offset,
                ap=[list(ap.ap[0]), [0, width]],
            )

        # ---- constants ----
        ident = singles.tile([128, 128], f32)
        make_identity(nc, ident[:])

        iota_i = singles.tile([128, WIN], i32)
        nc.gpsimd.iota(iota_i[:], pattern=[[1, WIN]], channel_multiplier=0)
        iota_f = singles.tile([128, WIN], f32)
        nc.vector.tensor_copy(iota_f[:], iota_i[:])

        rmod = singles.tile([16, 128], i32)
        nc.gpsimd.iota(
            rmod[:].rearrange("p (a b) -> p a b", a=2),
            pattern=[[0, 2], [320, 64]],
            channel_multiplier=20,
        )

        ncl_sb = singles.tile([128, 1], i32)
        nc.sync.dma_start(out=ncl_sb[:], in_=ncl_ext[:])

        # ---- labels ----
        lab_pg = singles.tile([128, NG], i32)   # [p, g] = labels[g*128+p]
        nc.sync.dma_start(out=lab_pg[:], in_=lab_ext[:].rearrange("(g p) -> p g", p=128))
        lab16 = singles.tile([16, 128], i32)    # [p, s] = labels[s*16+p]
        nc.sync.dma_start(out=lab16[:], in_=lab_ext[:].rearrange("(s p) -> p s", p=16))

        # ---- pass A: row norms of x ----
        ss16 = singles.tile([128, NG], f32)
        for g in range(NG):
            x_t = xt_pool.tile([128, D], f32, tag="xA")
            nc.sync.dma_start(out=x_t[:], in_=x_ext[g * 128:(g + 1) * 128, :])
            dmp = dump_pool.tile([128, D], f32, tag="sqdump")
            nc.scalar.activation(dmp[:], x_t[:], Act.Square,
                                 accum_out=ss16[:, g:g + 1])
        nrm16 = singles.tile([128, NG], f32)
        nc.scalar.activation(nrm16[:], ss16[:], Act.Sqrt)
        nc.vector.tensor_scalar_max(nrm16[:], nrm16[:], 1e-12)
        inv16 = singles.tile([128, NG], f32)
        nc.vector.reciprocal(inv16[:], nrm16[:])

        # ---- pass B: xn = x * inv_norm, transpose -> xnT (4 x [128, N]) ----
        xnT = [singles.tile([128, N], f32, tag=f"xnT{k}", name=f"xnT{k}") for k in range(4)]
        for g in range(NG):
            x_t = xt_pool.tile([128, D], f32, tag="xB")
            nc.sync.dma_start(out=x_t[:], in_=x_ext[g * 128:(g + 1) * 128, :])
            xn_t = xt_pool.tile([128, D], f32, tag="xn")
            nc.scalar.activation(xn_t[:], x_t[:], Act.Copy,
                                 scale=inv16[:, g:g + 1])
            for k in range(4):
                pt = ps_t.tile([128, 128], f32)
                nc.tensor.transpose(pt[:], xn_t[:, k * 128:(k + 1) * 128], ident[:])
                nc.vector.tensor_copy(xnT[k][:, g * 128:(g + 1) * 128], pt[:])

        # ---- W shard load + transpose -> wT (4 x [128, CPAD]) ----
        wT = [singles.tile([128, CPAD], f32, tag=f"wT{k}", name=f"wT{k}") for k in range(4)]
        NCT = CPAD // 128  # 10 column tiles
        for ct in range(NCT):
            w_t = wt_pool.tile([128, D], f32, tag="wld")
            rows = min(CSH - ct * 128, 128)
            if rows < 128:
                nc.vector.memset(w_t[:], 0.0)
            nc.sync.dma_start(out=w_t[:rows, :],
                              in_=w_ext[ct * 128: ct * 128 + rows, :])
            for k in range(4):
                pt = ps_t.tile([128, 128], f32)
                nc.tensor.transpose(pt[:], w_t[:, k * 128:(k + 1) * 128], ident[:])
                nc.vector.tensor_copy(wT[k][:, ct * 128:(ct + 1) * 128], pt[:])

        # ---- main loop: matmul + exp-sum + wf writeback ----
        esum48 = singles.tile([128, NG * 3], f32)
        wf_dmas = []
        for g in range(NG):
            for ci in range(3):
                cw = CW[ci]
                cv = CVALID[ci]
                c0 = ci * 512
                pm = ps_mm.tile([128, 512], f32, tag="mm")
                for k in range(4):
                    nc.tensor.matmul(
                        pm[:, :cw],
                        lhsT=xnT[k][:, g * 128:(g + 1) * 128],
                        rhs=wT[k][:, c0:c0 + cw],
                        start=(k == 0),
                        stop=(k == 3),
                    )
                wf_t = wf_pool.tile([128, 512], f32, tag="wf")
                nc.vector.tensor_copy(wf_t[:, :cw], pm[:, :cw])
                dmp = dump_pool.tile([128, 512], f32, tag="expdump")
                nc.scalar.activation(dmp[:, :cv], pm[:, :cv], Act.Exp,
                                     scale=S_SCALE,
                                     accum_out=esum48[:, g * 3 + ci:g * 3 + ci + 1])
                dma = nc.sync.dma_start(
                    out=wf_ext[g * 128:(g + 1) * 128, c0:c0 + cw],
                    in_=wf_t[:, :cw])
                wf_dmas.append(dma)

        partials = singles.tile([128, 48], f32)
        nc.vector.tensor_reduce(
            partials[:, 0:NG],
            esum48[:].rearrange("p (g c) -> p g c", c=3),
            axis=mybir.AxisListType.X, op=Alu.add)

        # ---- label-window bookkeeping (int math) ----
        # local label l = y - c_lo ; block b = clamp((l >> 6) - 1, 0, 17)
        lpg = singles.tile([128, NG], i32)
        nc.vector.tensor_tensor(lpg[:], lab_pg[:], bcast_col(ncl_sb[:, 0:1], NG),
                                op=Alu.add)
        bpg = singles.tile([128, NG], i32)
        nc.vector.tensor_scalar(bpg[:], lpg[:], 6, None, op0=Alu.arith_shift_right)
        nc.vector.tensor_scalar(bpg[:], bpg[:], -1, 0, op0=Alu.add, op1=Alu.max)
        nc.vector.tensor_scalar(bpg[:], bpg[:], BMAX, None, op0=Alu.min)
        apg = singles.tile([128, NG], i32)
        nc.vector.tensor_scalar_mul(apg[:], bpg[:], BLK)
        af = singles.tile([128, NG], f32)
        nc.vector.tensor_copy(af[:], apg[:])
        s0i = singles.tile([128, NG], i32)
        nc.vector.tensor_tensor(s0i[:], apg[:], lpg[:], op=Alu.subtract)
        s0f = singles.tile([128, NG], f32)
        nc.vector.tensor_copy(s0f[:], s0i[:])

        # gather indices (int16, [channels=16] layout, value = 20*r_local + b)
        l16 = singles.tile([16, 128], i32)
        nc.vector.tensor_tensor(l16[:], lab16[:], bcast_col(ncl_sb[:16, 0:1], 128),
                                op=Alu.add)
        b16 = singles.tile([16, 128], i32)
        nc.vector.tensor_scalar(b16[:], l16[:], 6, None, op0=Alu.arith_shift_right)
        nc.vector.tensor_scalar(b16[:], b16[:], -1, 0, op0=Alu.add, op1=Alu.max)
        nc.vector.tensor_scalar(b16[:], b16[:], BMAX, None, op0=Alu.min)
        idx32 = singles.tile([16, 128], i32)
        nc.vector.tensor_tensor(idx32[:], b16[:], rmod[:], op=Alu.add)
        idx16 = singles.tile([128, 128], i16)
        nc.vector.memset(idx16[:], 0)
        nc.vector.tensor_copy(idx16[:16, :], idx32[:])

        # ---- windowed soft-label term ----
        HALF_ROWS = 1024
        VROWS = (HALF_ROWS - 1) * (CPAD // BLK) + BMAX + 1  # 20478
        for h in range(2):
            win_t = work.tile([128, 8 * WIN], f32, tag="win")
            gth = nc.gpsimd.dma_gather(
                out_ap=win_t[:].rearrange("p (g w) -> p g w", w=WIN),
                in_ap=bass.AP(tensor=wf_ext, offset=h * HALF_ROWS * CPAD,
                              ap=[[BLK, VROWS], [1, WIN]]),
                idxs_ap=idx16[:, h * 64:(h + 1) * 64],
                num_idxs=HALF_ROWS,
                num_idxs_reg=HALF_ROWS,
                elem_size=WIN,
                elem_step=BLK,
            )
            from concourse.bass import _add_dep_helper
            for dma in wf_dmas:
                _add_dep_helper(gth.ins, dma.ins, True, "gather after wf writeback")

            w3 = win_t[:].rearrange("p (g w) -> p g w", w=WIN)
            s0_b = bcast(s0f[:, h * 8:(h + 1) * 8], 8, WIN)
            a_b = bcast(af[:, h * 8:(h + 1) * 8], 8, WIN)
            iota_b = bass.AP(tensor=iota_f.tensor, offset=iota_f.offset,
                             ap=[list(iota_f.ap[0]), [0, 8], [1, WIN]])

            d_t = work.tile([128, 8 * WIN], f32, tag="d")
            d3 = d_t[:].rearrange("p (g w) -> p g w", w=WIN)
            nc.vector.tensor_tensor(d3, iota_b, s0_b, op=Alu.add)
            # kern = exp(-0.2*|d|), masked to local col < CSH; in-place chain
            kern = work.tile([128, 8 * WIN], f32, tag="kern")
            nc.vector.scalar_tensor_tensor(
                kern[:], d_t[:], -1.0, d_t[:], op0=Alu.mult, op1=Alu.max)
            nc.scalar.activation(kern[:], kern[:], Act.Exp, scale=-KDECAY)
            lcol = work.tile([128, 8 * WIN], f32, tag="lcol")
            nc.vector.tensor_tensor(
                lcol[:].rearrange("p (g w) -> p g w", w=WIN), iota_b, a_b, op=Alu.add)
            nc.vector.scalar_tensor_tensor(
                kern[:], lcol[:], float(CSH), kern[:],
                op0=Alu.is_lt, op1=Alu.mult)
            tcl = work.tile([128, 8 * WIN], f32, tag="tcl")
            nc.vector.tensor_scalar(tcl[:], win_t[:], -CLIP, CLIP,
                                    op0=Alu.max, op1=Alu.min)
            # u = (A/B')*t + sqrt(1-t^2), built in-place in one buffer
            u_t = work.tile([128, 8 * WIN], f32, tag="u")
            nc.vector.tensor_tensor(u_t[:], tcl[:], tcl[:], op=Alu.mult)
            nc.scalar.activation(u_t[:], u_t[:], Act.Sqrt, scale=-1.0, bias=1.0)
            nc.vector.scalar_tensor_tensor(
                u_t[:], tcl[:], A_OVER_B, u_t[:], op0=Alu.mult, op1=Alu.add)
            nc.vector.tensor_tensor(kern[:], kern[:], u_t[:], op=Alu.mult)
            nc.vector.tensor_reduce(
                partials[:, 16 + 8 * h:16 + 8 * (h + 1)],
                kern[:].rearrange("p (g w) -> p g w", w=WIN),
                axis=mybir.AxisListType.X, op=Alu.add)
            nc.vector.scalar_tensor_tensor(
                win_t[:], d_t[:], 0.0, win_t[:], op0=Alu.is_equal, op1=Alu.mult)
            nc.vector.tensor_reduce(
                partials[:, 32 + 8 * h:32 + 8 * (h + 1)],
                win_t[:].rearrange("p (g w) -> p g w", w=WIN),
                axis=mybir.AxisListType.X, op=Alu.add)

        # ---- AllReduce partials ----
        nc.sync.dma_start(out=ar_in[:].rearrange("(p f) -> p f", p=128),
                          in_=partials[:])
        nc.gpsimd.collective_compute(
            "AllReduce", mybir.AluOpType.add,
            replica_groups=[list(range(NCORES))],
            ins=[ar_in[:]], outs=[ar_out[:]])
        red = singles.tile([128, 48], f32)
        nc.sync.dma_start(out=red[:], in_=ar_out[:].rearrange("(p f) -> p f", p=128))
        esumT = red[:, 0:16]
        t1T = red[:, 16:32]
        wfyT = red[:, 32:48]

        # ---- normalization Z (analytic geometric sums) ----
        yf = singles.tile([128, NG], f32)
        nc.vector.tensor_copy(yf[:], lab_pg[:])
        mm_t = singles.tile([128, NG], f32)
        nc.vector.tensor_scalar(mm_t[:], yf[:], float(C - 1) - 0.0, None,
                                op0=Alu.min)  # min(y, 4999)? no: compute both
        # mL+1 = min(y, 4999) + 1
        mL = singles.tile([128, NG], f32)
        nc.vector.tensor_scalar(mL[:], yf[:], float((C // 2) - 1), 1.0,
                                op0=Alu.min, op1=Alu.add)
        gL = singles.tile([128, NG], f32)
        nc.scalar.activation(gL[:], mL[:], Act.Exp, scale=-KDECAY)
        # mR+1 = min(C-1-y, 5000) + 1
        mR = singles.tile([128, NG], f32)
        nc.vector.tensor_scalar(mR[:], yf[:], -1.0, float(C - 1),
                                op0=Alu.mult, op1=Alu.add)
        nc.vector.tensor_scalar(mR[:], mR[:], float(C // 2), 1.0,
                                op0=Alu.min, op1=Alu.add)
        gR = singles.tile([128, NG], f32)
        nc.scalar.activation(gR[:], mR[:], Act.Exp, scale=-KDECAY)
        z_t = singles.tile([128, NG], f32)
        nc.vector.tensor_tensor(z_t[:], gL[:], gR[:], op=Alu.add)
        nc.vector.tensor_scalar(z_t[:], z_t[:], -C2, C1, op0=Alu.mult, op1=Alu.add)
        invz = singles.tile([128, NG], f32)
        nc.vector.reciprocal(invz[:], z_t[:])

        # ---- final per-row loss ----
        tcy = singles.tile([128, NG], f32)
        nc.vector.tensor_scalar(tcy[:], wfyT, -CLIP, CLIP, op0=Alu.max, op1=Alu.min)
        ty2 = singles.tile([128, NG], f32)
        nc.vector.tensor_tensor(ty2[:], tcy[:], tcy[:], op=Alu.mult)
        s2y = singles.tile([128, NG], f32)
        nc.scalar.activation(s2y[:], ty2[:], Act.Sqrt, scale=-1.0, bias=1.0)
        tyA = singles.tile([128, NG], f32)
        nc.vector.tensor_scalar_mul(tyA[:], tcy[:], A_COS)
        numy = singles.tile([128, NG], f32)
        nc.vector.scalar_tensor_tensor(
            numy[:], s2y[:], B_SIN, tyA[:], op0=Alu.mult, op1=Alu.add)
        eny = singles.tile([128, NG], f32)
        nc.scalar.activation(eny[:], numy[:], Act.Exp)
        ey = singles.tile([128, NG], f32)
        nc.scalar.activation(ey[:], wfyT, Act.Exp, scale=S_SCALE)
        den = singles.tile([128, NG], f32)
        nc.vector.tensor_tensor(den[:], eny[:], esumT, op=Alu.add)
        nc.vector.tensor_tensor(den[:], den[:], ey[:], op=Alu.subtract)
        lden = singles.tile([128, NG], f32)
        nc.scalar.activation(lden[:], den[:], Act.Ln)
        q1 = singles.tile([128, NG], f32)
        nc.vector.tensor_tensor(q1[:], t1T, invz[:], op=Alu.mult)
        l_t = singles.tile([128, NG], f32)
        nc.vector.scalar_tensor_tensor(
            l_t[:], q1[:], B_SIN, lden[:], op0=Alu.mult, op1=Alu.subtract)
        nc.sync.dma_start(out=l_ext[:].rearrange("(g p) -> p g", p=128), in_=l_t[:])

    nc.finalize()
    return nc


def _get_nc():
    if "nc" not in _CACHE:
        _CACHE["nc"] = _build()
    return _CACHE["nc"]


def kernel(x, labels, W):
    from concourse.bass_utils import run_bass_kernel_spmd

    nc = _get_nc()
    x = np.ascontiguousarray(x, dtype=np.float32)
    W = np.ascontiguousarray(W, dtype=np.float32)
    labels = np.ascontiguousarray(labels, dtype=np.int32)
    in_maps = []
    for i in range(NCORES):
        in_maps.append({
            "x": x,
            "w": np.ascontiguousarray(W[i * CSH:(i + 1) * CSH]),
            "labels": labels,
            "negclo": np.full((128, 1), -i * CSH, dtype=np.int32),
        })
    res = run_bass_kernel_spmd(nc, in_maps, core_ids=list(range(NCORES)))
    wf = np.concatenate(
        [res.results[i]["wf_out"][:, :CSH] for i in range(NCORES)], axis=1)
    loss = np.float32(-np.mean(res.results[0]["l_out"]))
    return wf, loss


# revision 15
# speedup vs baseline: 1.4295x; 1.4295x over previous
"""Trainium2 Bass kernel for AngularPenaltySMLossWithSoftLabel.

Strategy: tensor-parallel over out_features C=10000 across 8 cores
(1250 columns each). Each core:
  - transposes its W shard and the full (raw) x on the PE,
  - computes wf[:, c_lo:c_hi] = x @ W_shard.T in float32r, scaling rows
    by 1/||x_n|| in the epilogue (copy + fused exp-sum accumulate),
  - gathers a 192-wide window around each row's label (clipped to its
    shard) from its wf shard via indirect DMA and computes the
    soft-label kernel * arcface-numerator contribution (the
    exp(-0.2*d) kernel is < 3e-6 outside +-64),
  - ReduceScatters [exp_sum, term1, wf_y] (3 x 2048 f32) across cores,
  - computes the final per-row loss slice L (256 rows/core) on device.
Host glue: concatenate wf shards, slice padding, loss = -mean(L).
"""

import math
import numpy as np

N, D, C = 2048, 512, 10000
NCORES = 8
CSH = C // NCORES          # 1250 columns per core
CPAD = 1280                # padded row stride of the wf output (256B multiple)
S_SCALE = 64.0
MARGIN = 0.5
EPS = 1e-7
KDECAY = 0.2

A_COS = S_SCALE * math.cos(MARGIN)            # 56.1652...
B_SIN = -S_SCALE * math.sin(MARGIN)           # -30.6832...
A_OVER_B = math.cos(MARGIN) / -math.sin(MARGIN)  # -1.8304877...
CLIP = 1.0 - EPS
G = math.exp(-KDECAY)
C2 = 1.0 / (1.0 - G)                          # 5.51667
C1 = 1.0 + 2.0 * G * C2                       # 10.03331

WIN = 192          # gather window width (64-aligned, covers label +-64)
BLK = 64           # gather stride granularity (256B in f32)
BMAX = (CPAD - WIN) // BLK   # 17: max window block start
NG = N // 128      # 16 row-chunks of 128
CW = [512, 512, 256]         # c-chunk widths (local columns)
CVALID = [512, 512, CSH - 1024]  # valid (non-ghost) widths

_CACHE = {}


def _build():
    import concourse.bass as bass
    import concourse.mybir as mybir
    from concourse import bacc, tile
    from concourse.masks import make_identity
    from concourse.bass import _add_dep_helper
    from contextlib import ExitStack

    f32 = mybir.dt.float32
    f32r = mybir.dt.float32r
    i32 = mybir.dt.int32
    i16 = mybir.dt.int16
    Alu = mybir.AluOpType
    Act = mybir.ActivationFunctionType

    nc = bacc.Bacc("TRN2", num_devices=NCORES)

    x_ext = nc.declare_dram_parameter("x", [N, D], f32, isOutput=False)
    w_ext = nc.declare_dram_parameter("w", [CSH, D], f32, isOutput=False)
    lab_ext = nc.declare_dram_parameter("labels", [N], i32, isOutput=False)
    ncl_ext = nc.declare_dram_parameter("negclo", [128, 1], i32, isOutput=False)
    lsl_ext = nc.declare_dram_parameter("labslice", [16, 16], i32, isOutput=False)
    wf_ext = nc.declare_dram_parameter("wf_out", [N, CPAD], f32, isOutput=True)
    l_ext = nc.declare_dram_parameter("l_out", [256], f32, isOutput=True)

    ar_in = nc.dram_tensor("ar_in", [128 * 48], f32)
    rs_out = nc.dram_tensor("rs_out", [16 * 48], f32)

    with ExitStack() as ctx:
        tc = ctx.enter_context(tile.TileContext(nc))
        singles = ctx.enter_context(tc.tile_pool(name="singles", bufs=1))
        xt_pool = ctx.enter_context(tc.tile_pool(name="xt", bufs=3))
        wt_pool = ctx.enter_context(tc.tile_pool(name="wt", bufs=3))
        ps_t = ctx.enter_context(tc.tile_pool(name="pst", bufs=4, space="PSUM"))
        ps_mm = ctx.enter_context(tc.tile_pool(name="psmm", bufs=4, space="PSUM"))
        wf_pool = ctx.enter_context(tc.tile_pool(name="wfout", bufs=4))
        dump_pool = ctx.enter_context(tc.tile_pool(name="edump", bufs=2))
        work = ctx.enter_context(tc.tile_pool(name="work", bufs=1))

        def bc3(ap, width):
            """[P, k] slice -> [P, k, width] broadcast (stride-0 inner)."""
            return bass.AP(tensor=ap.tensor, offset=ap.offset,
                           ap=[list(ap.ap[0]), list(ap.ap[1]), [0, width]])

        def bc2(ap, width):
            """[P, 1] column -> [P, width] broadcast."""
            return bass.AP(tensor=ap.tensor, offset=ap.offset,
                           ap=[list(ap.ap[0]), [0, width]])

        # ---- constants ----
        ident = singles.tile([128, 128], f32)
        make_identity(nc, ident[:])

        iota_i = singles.tile([128, WIN], i32)
        nc.gpsimd.iota(iota_i[:], pattern=[[1, WIN]], channel_multiplier=0)
        iota_f = singles.tile([128, WIN], f32)
        nc.vector.tensor_copy(iota_f[:], iota_i[:])
        iota_b = bass.AP(tensor=iota_f.tensor, offset=iota_f.offset,
                         ap=[list(iota_f[:].ap[0]), [0, 8], [1, WIN]])

        rmod = singles.tile([16, 128], i32)
        nc.gpsimd.iota(rmod[:].rearrange("p (a b) -> p a b", a=2),
                       pattern=[[0, 2], [320, 64]], channel_multiplier=20)

        ncl_sb = singles.tile([128, 1], i32)
        nc.sync.dma_start(out=ncl_sb[:], in_=ncl_ext[:])
        lab_pg = singles.tile([128, NG], i32)   # [p, g] = labels[g*128+p]
        nc.sync.dma_start(out=lab_pg[:],
                          in_=lab_ext[:].rearrange("(g p) -> p g", p=128))
        lab16 = singles.tile([16, 128], i32)    # [p, s] = labels[s*16+p]
        nc.sync.dma_start(out=lab16[:],
                          in_=lab_ext[:].rearrange("(s p) -> p s", p=16))
        labsl = singles.tile([16, 16], i32)
        nc.sync.dma_start(out=labsl[:], in_=lsl_ext[:])

        # ---- W shard load + transpose -> wT (4 x [128, CPAD]) ----
        wT = [singles.tile([128, CPAD], f32r, tag=f"wT{k}", name=f"wT{k}")
              for k in range(4)]
        for ct in range(CPAD // 128):
            w_t = wt_pool.tile([128, D], f32, tag="wld")
            rows = min(CSH - ct * 128, 128)
            if rows < 128:
                nc.vector.memset(w_t[:], 0.0)
            nc.sync.dma_start(out=w_t[:rows, :],
                              in_=w_ext[ct * 128: ct * 128 + rows, :])
            for k in range(4):
                pt = ps_t.tile([128, 128], f32)
                nc.tensor.transpose(pt[:], w_t[:, k * 128:(k + 1) * 128], ident[:])
                nc.vector.tensor_copy(wT[k][:, ct * 128:(ct + 1) * 128], pt[:])

        # ---- x load: row sq-norms (ACT) + raw transpose (PE) in parallel ----
        xnT = [singles.tile([128, N], f32r, tag=f"xnT{k}", name=f"xnT{k}")
               for k in range(4)]
        ss16 = singles.tile([128, NG], f32)
        for g in range(NG):
            x_t = xt_pool.tile([128, D], f32, tag="xA")
            nc.sync.dma_start(out=x_t[:], in_=x_ext[g * 128:(g + 1) * 128, :])
            dmp = dump_pool.tile([128, D], f32, tag="sqdump")
            nc.scalar.activation(dmp[:], x_t[:], Act.Square,
                                 accum_out=ss16[:, g:g + 1])
            for k in range(4):
                pt = ps_t.tile([128, 128], f32)
                nc.tensor.transpose(pt[:], x_t[:, k * 128:(k + 1) * 128], ident[:])
                nc.vector.tensor_copy(xnT[k][:, g * 128:(g + 1) * 128], pt[:])
        inv16 = singles.tile([128, NG], f32)
        nc.scalar.activation(inv16[:], ss16[:], Act.Sqrt)
        nc.vector.tensor_scalar_max(inv16[:], inv16[:], 1e-12)
        nc.vector.reciprocal(inv16[:], inv16[:])
        sinv16 = singles.tile([128, NG], f32)
        nc.vector.tensor_scalar_mul(sinv16[:], inv16[:], S_SCALE)

        # ---- label-window bookkeeping (int math) ----
        lpg = singles.tile([128, NG], i32)
        nc.vector.tensor_tensor(lpg[:], lab_pg[:], bc2(ncl_sb[:, 0:1], NG),
                                op=Alu.add)
        bpg = singles.tile([128, NG], i32)
        nc.vector.tensor_scalar(bpg[:], lpg[:], 6, None, op0=Alu.arith_shift_right)
        nc.vector.tensor_scalar(bpg[:], bpg[:], -1, 0, op0=Alu.add, op1=Alu.max)
        nc.vector.tensor_scalar(bpg[:], bpg[:], BMAX, None, op0=Alu.min)
        apg = singles.tile([128, NG], i32)
        nc.vector.tensor_scalar_mul(apg[:], bpg[:], BLK)
        af = singles.tile([128, NG], f32)
        nc.vector.tensor_copy(af[:], apg[:])
        s0i = singles.tile([128, NG], i32)
        nc.vector.tensor_tensor(s0i[:], apg[:], lpg[:], op=Alu.subtract)
        s0f = singles.tile([128, NG], f32)
        nc.vector.tensor_copy(s0f[:], s0i[:])

        l16 = singles.tile([16, 128], i32)
        nc.vector.tensor_tensor(l16[:], lab16[:], bc2(ncl_sb[:16, 0:1], 128),
                                op=Alu.add)
        b16 = singles.tile([16, 128], i32)
        nc.vector.tensor_scalar(b16[:], l16[:], 6, None, op0=Alu.arith_shift_right)
        nc.vector.tensor_scalar(b16[:], b16[:], -1, 0, op0=Alu.add, op1=Alu.max)
        nc.vector.tensor_scalar(b16[:], b16[:], BMAX, None, op0=Alu.min)
        idx32 = singles.tile([16, 128], i32)
        nc.vector.tensor_tensor(idx32[:], b16[:], rmod[:], op=Alu.add)
        idx16 = singles.tile([128, 128], i16)
        nc.vector.memset(idx16[:], 0)
        nc.vector.tensor_copy(idx16[:16, :], idx32[:])

        partials = singles.tile([128, 48], f32)
        esum48 = singles.tile([128, NG * 3], f32)

        HALF_ROWS = 1024
        VROWS = (HALF_ROWS - 1) * (CPAD // BLK) + BMAX + 1  # 20478

        def emit_gather(h, eng_win):
            win_t = work.tile([128, 8 * WIN], f32, tag=f"win{h}",
                              name=f"win{h}")
            gth = nc.gpsimd.dma_gather(
                out_ap=win_t[:].rearrange("p (g w) -> p g w", w=WIN),
                in_ap=bass.AP(tensor=wf_ext, offset=h * HALF_ROWS * CPAD,
                              ap=[[BLK, VROWS], [1, WIN]]),
                idxs_ap=idx16[:, h * 64:(h + 1) * 64],
                num_idxs=HALF_ROWS,
                num_idxs_reg=HALF_ROWS,
                elem_size=WIN,
                elem_step=BLK,
            )
            return win_t, gth

        def win_early(h, eng, win_t):
            """d, |d|, lcol, tcl, tt2 — only needs gathered win + labels."""
            s0_b = bc3(s0f[:, h * 8:(h + 1) * 8], WIN)
            a_b = bc3(af[:, h * 8:(h + 1) * 8], WIN)
            d_t = work.tile([128, 8 * WIN], f32, tag=f"d{h}", name=f"d{h}")
            nc_e = getattr(nc, eng)
            nc_e.tensor_tensor(d_t[:].rearrange("p (g w) -> p g w", w=WIN),
                               iota_b, s0_b, op=Alu.add)
            ad_t = work.tile([128, 8 * WIN], f32, tag=f"ad{h}", name=f"ad{h}")
            nc_e.scalar_tensor_tensor(ad_t[:], d_t[:], -1.0, d_t[:],
                                      op0=Alu.mult, op1=Alu.max)
            lc_t = work.tile([128, 8 * WIN], f32, tag=f"lc{h}", name=f"lc{h}")
            nc_e.tensor_tensor(lc_t[:].rearrange("p (g w) -> p g w", w=WIN),
                               iota_b, a_b, op=Alu.add)
            tcl = work.tile([128, 8 * WIN], f32, tag=f"tcl{h}", name=f"tcl{h}")
            nc_e.tensor_scalar(tcl[:], win_t[:], -CLIP, CLIP,
                               op0=Alu.max, op1=Alu.min)
            tt2 = work.tile([128, 8 * WIN], f32, tag=f"tt2{h}", name=f"tt2{h}")
            nc_e.tensor_tensor(tt2[:], tcl[:], tcl[:], op=Alu.mult)
            return d_t, ad_t, lc_t, tcl, tt2

        def win_act(h, ad_t, tt2):
            kern = work.tile([128, 8 * WIN], f32, tag=f"kern{h}", name=f"kern{h}")
            nc.scalar.activation(kern[:], ad_t[:], Act.Exp, scale=-KDECAY)
            s2_t = work.tile([128, 8 * WIN], f32, tag=f"s2{h}", name=f"s2{h}")
            nc.scalar.activation(s2_t[:], tt2[:], Act.Sqrt, scale=-1.0, bias=1.0)
            return kern, s2_t

        def win_late(h, eng, win_t, d_t, lc_t, tcl, kern, s2_t):
            """mask kern, u, c1; reduces go on DVE."""
            nc_e = getattr(nc, eng)
            nc_e.scalar_tensor_tensor(kern[:], lc_t[:], float(CSH), kern[:],
                                      op0=Alu.is_lt, op1=Alu.mult)
            nc_e.scalar_tensor_tensor(s2_t[:], tcl[:], A_OVER_B, s2_t[:],
                                      op0=Alu.mult, op1=Alu.add)
            nc_e.tensor_tensor(kern[:], kern[:], s2_t[:], op=Alu.mult)
            nc_e.scalar_tensor_tensor(win_t[:], d_t[:], 0.0, win_t[:],
                                      op0=Alu.is_equal, op1=Alu.mult)

        def win_reduce(h, win_t, kern):
            nc.vector.tensor_reduce(
                partials[:, 16 + 8 * h:16 + 8 * (h + 1)],
                kern[:].rearrange("p (g w) -> p g w", w=WIN),
                axis=mybir.AxisListType.X, op=Alu.add)
            nc.vector.tensor_reduce(
                partials[:, 32 + 8 * h:32 + 8 * (h + 1)],
                win_t[:].rearrange("p (g w) -> p g w", w=WIN),
                axis=mybir.AxisListType.X, op=Alu.add)

        # ---- main loop: f32r matmul + scaled copy + exp-sum + writeback ----
        wf_dmas = []
        win_state = {}
        for g in range(NG):
            for ci in range(3):
                cw, cv, c0 = CW[ci], CVALID[ci], ci * 512
                pm = ps_mm.tile([128, 512], f32, tag="mm")
                for k in range(4):
                    nc.tensor.matmul(
                        pm[:, :cw],
                        lhsT=xnT[k][:, g * 128:(g + 1) * 128],
                        rhs=wT[k][:, c0:c0 + cw],
                        start=(k == 0), stop=(k == 3))
                wf_t = wf_pool.tile([128, 512], f32, tag="wf")
                nc.vector.tensor_scalar_mul(wf_t[:, :cw], pm[:, :cw],
                                            inv16[:, g:g + 1])
                dmp = dump_pool.tile([128, 512], f32, tag="expdump")
                nc.scalar.activation(dmp[:, :cv], pm[:, :cv], Act.Exp,
                                     scale=sinv16[:, g:g + 1],
                                     accum_out=esum48[:, g * 3 + ci:g * 3 + ci + 1])
                dma = nc.sync.dma_start(
                    out=wf_ext[g * 128:(g + 1) * 128, c0:c0 + cw],
                    in_=wf_t[:, :cw])
                wf_dmas.append(dma)

            if g == 7:
                win0, gth0 = emit_gather(0, "gpsimd")
                for dma in wf_dmas[:24]:
                    _add_dep_helper(gth0.ins, dma.ins, True, "gather0 after wf g0-7")
            if g == 12:
                w0_early = win_early(0, "vector", win0)
            if g == 13:
                d0, ad0, lc0, tcl0, tt20 = w0_early
                kern0, s20 = win_act(0, ad0, tt20)
            if g == 14:
                win_late(0, "vector", win0, d0, lc0, tcl0, kern0, s20)

        win_reduce(0, win0, kern0)

        # ---- second-half window phase (post-loop, DVE has idle time) ----
        win1, gth1 = emit_gather(1, "vector")
        for dma in wf_dmas[24:]:
            _add_dep_helper(gth1.ins, dma.ins, True, "gather1 after wf g8-15")
        d1, ad1, lc1, tcl1, tt21 = win_early(1, "vector", win1)
        kern1, s21 = win_act(1, ad1, tt21)
        win_late(1, "vector", win1, d1, lc1, tcl1, kern1, s21)
        win_reduce(1, win1, kern1)

        # exp-sum reduction into partials
        nc.vector.tensor_reduce(
            partials[:, 0:NG],
            esum48[:].rearrange("p (g c) -> p g c", c=3),
            axis=mybir.AxisListType.X, op=Alu.add)

        # ---- ReduceScatter partials: core i gets rows [16i, 16i+16) ----
        nc.sync.dma_start(out=ar_in[:].rearrange("(p f) -> p f", p=128),
                          in_=partials[:])
        nc.gpsimd.collective_compute(
            "ReduceScatter", mybir.AluOpType.add,
            replica_groups=[list(range(NCORES))],
            ins=[ar_in[:]], outs=[rs_out[:]])
        red = singles.tile([16, 48], f32)
        nc.sync.dma_start(out=red[:], in_=rs_out[:].rearrange("(p f) -> p f", p=16))
        esumT = red[:, 0:16]
        t1T = red[:, 16:32]
        wfyT = red[:, 32:48]

        # ---- normalization Z (analytic geometric sums), [16,16] slice ----
        yf = singles.tile([16, 16], f32)
        nc.vector.tensor_copy(yf[:], labsl[:])
        mL = singles.tile([16, 16], f32)
        nc.vector.tensor_scalar(mL[:], yf[:], float((C // 2) - 1), 1.0,
                                op0=Alu.min, op1=Alu.add)
        gL = singles.tile([16, 16], f32)
        nc.scalar.activation(gL[:], mL[:], Act.Exp, scale=-KDECAY)
        mR = singles.tile([16, 16], f32)
        nc.vector.tensor_scalar(mR[:], yf[:], -1.0, float(C - 1),
                                op0=Alu.mult, op1=Alu.add)
        nc.vector.tensor_scalar(mR[:], mR[:], float(C // 2), 1.0,
                                op0=Alu.min, op1=Alu.add)
        gR = singles.tile([16, 16], f32)
        nc.scalar.activation(gR[:], mR[:], Act.Exp, scale=-KDECAY)
        z_t = singles.tile([16, 16], f32)
        nc.vector.tensor_tensor(z_t[:], gL[:], gR[:], op=Alu.add)
        nc.vector.tensor_scalar(z_t[:], z_t[:], -C2, C1, op0=Alu.mult, op1=Alu.add)
        invz = singles.tile([16, 16], f32)
        nc.vector.reciprocal(invz[:], z_t[:])

        # ---- final per-row loss slice ----
        tcy = singles.tile([16, 16], f32)
        nc.vector.tensor_scalar(tcy[:], wfyT, -CLIP, CLIP, op0=Alu.max, op1=Alu.min)
        ty2 = singles.tile([16, 16], f32)
        nc.vector.tensor_tensor(ty2[:], tcy[:], tcy[:], op=Alu.mult)
        s2y = singles.tile([16, 16], f32)
        nc.scalar.activation(s2y[:], ty2[:], Act.Sqrt, scale=-1.0, bias=1.0)
        tyA = singles.tile([16, 16], f32)
        nc.vector.tensor_scalar_mul(tyA[:], tcy[:], A_COS)
        numy = singles.tile([16, 16], f32)
        nc.vector.scalar_tensor_tensor(numy[:], s2y[:], B_SIN, tyA[:],
                                       op0=Alu.mult, op1=Alu.add)
        eny = singles.tile([16, 16], f32)
        nc.scalar.activation(eny[:], numy[:], Act.Exp)
        ey = singles.tile([16, 16], f32)
        nc.scalar.activation(ey[:], wfyT, Act.Exp, scale=S_SCALE)
        den = singles.tile([16, 16], f32)
        nc.vector.tensor_tensor(den[:], eny[:], esumT, op=Alu.add)
        nc.vector.tensor_tensor(den[:], den[:], ey[:], op=Alu.subtract)
        lden = singles.tile([16, 16], f32)
        nc.scalar.activation(lden[:], den[:], Act.Ln)
        q1 = singles.tile([16, 16], f32)
        nc.vector.tensor_tensor(q1[:], t1T, invz[:], op=Alu.mult)
        l_t = singles.tile([16, 16], f32)
        nc.vector.scalar_tensor_tensor(l_t[:], q1[:], B_SIN, lden[:],
                                       op0=Alu.mult, op1=Alu.subtract)
        nc.sync.dma_start(out=l_ext[:].rearrange("(pp g) -> pp g", g=16),
                          in_=l_t[:])

    nc.finalize()
    return nc


def _get_nc():
    if "nc" not in _CACHE:
        _CACHE["nc"] = _build()
    return _CACHE["nc"]


def make_in_maps(x, labels, W):
    x = np.ascontiguousarray(x, dtype=np.float32)
    W = np.ascontiguousarray(W, dtype=np.float32)
    labels = np.ascontiguousarray(labels, dtype=np.int32)
    lab2d = labels.reshape(NG, 128)  # [g, p]
    in_maps = []
    for i in range(NCORES):
        # labslice[pp, g] = labels[g*128 + 16*i + pp]
        lsl = np.ascontiguousarray(lab2d[:, 16 * i:16 * (i + 1)].T)
        in_maps.append({
            "x": x,
            "w": np.ascontiguousarray(W[i * CSH:(i + 1) * CSH]),
            "labels": labels,
            "negclo": np.full((128, 1), -i * CSH, dtype=np.int32),
            "labslice": lsl,
        })
    return in_maps


def assemble(outs):
    """outs: list of dicts with 'wf_out' [N, CPAD] and 'l_out' [256]."""
    wf = np.concatenate([outs[i]["wf_out"][:, :CSH] for i in range(NCORES)],
                        axis=1)
    l2d = np.zeros((NG, 128), dtype=np.float32)  # [g, p]
    for i in range(NCORES):
        l2d[:, 16 * i:16 * (i + 1)] = outs[i]["l_out"].reshape(16, 16).T
    loss = np.float32(-np.mean(l2d))
    return wf, loss


def kernel(x, labels, W):
    from concourse.bass_utils import run_bass_kernel_spmd

    nc = _get_nc()
    in_maps = make_in_maps(x, labels, W)
    res = run_bass_kernel_spmd(nc, in_maps, core_ids=list(range(NCORES)))
    return assemble(res.results)


# revision 21
# speedup vs baseline: 1.5753x; 1.1020x over previous
"""Trainium2 Bass kernel for AngularPenaltySMLossWithSoftLabel.

Strategy: tensor-parallel over out_features C=10000 across 8 cores
(1250 columns each). Each core:
  - transposes its W shard and the full (raw) x on the PE,
  - computes wf[:, c_lo:c_hi] = x @ W_shard.T in float32r, scaling rows
    by 1/||x_n|| in the epilogue (copy + fused exp-sum accumulate),
  - gathers a 192-wide window around each row's label (clipped to its
    shard) from its wf shard via indirect DMA and computes the
    soft-label kernel * arcface-numerator contribution (the
    exp(-0.2*d) kernel is < 3e-6 outside +-64),
  - ReduceScatters [exp_sum, term1, wf_y] (3 x 2048 f32) across cores,
  - computes the final per-row loss slice L (256 rows/core) on device.
Host glue: concatenate wf shards, slice padding, loss = -mean(L).
"""

import math
import numpy as np

N, D, C = 2048, 512, 10000
NCORES = 8
CSH = C // NCORES          # 1250 columns per core
CPAD = 1280                # padded row stride of the wf output (256B multiple)
S_SCALE = 64.0
MARGIN = 0.5
EPS = 1e-7
KDECAY = 0.2

A_COS = S_SCALE * math.cos(MARGIN)            # 56.1652...
B_SIN = -S_SCALE * math.sin(MARGIN)           # -30.6832...
A_OVER_B = math.cos(MARGIN) / -math.sin(MARGIN)  # -1.8304877...
CLIP = 1.0 - EPS
G = math.exp(-KDECAY)
C2 = 1.0 / (1.0 - G)                          # 5.51667
C1 = 1.0 + 2.0 * G * C2                       # 10.03331

WIN = 192          # gather window width (64-aligned, covers label +-64)
BLK = 64           # gather stride granularity (256B in f32)
BMAX = (CPAD - WIN) // BLK   # 17: max window block start
NG = N // 128      # 16 row-chunks of 128
CW = [512, 512, 256]         # c-chunk widths (local columns)
CVALID = [512, 512, CSH - 1024]  # valid (non-ghost) widths

_CACHE = {}


def _build():
    import concourse.bass as bass
    import concourse.mybir as mybir
    from concourse import bacc, tile
    from concourse.masks import make_identity
    from concourse.bass import _add_dep_helper
    from contextlib import ExitStack

    f32 = mybir.dt.float32
    f32r = mybir.dt.float32r
    bf16 = mybir.dt.bfloat16
    i32 = mybir.dt.int32
    i16 = mybir.dt.int16
    Alu = mybir.AluOpType
    Act = mybir.ActivationFunctionType

    nc = bacc.Bacc("TRN2", num_devices=NCORES)

    x_ext = nc.declare_dram_parameter("x", [N, D], f32, isOutput=False)
    w_ext = nc.declare_dram_parameter("w", [CSH, D], f32, isOutput=False)
    lab_ext = nc.declare_dram_parameter("labels", [N], i32, isOutput=False)
    ncl_ext = nc.declare_dram_parameter("negclo", [128, 1], i32, isOutput=False)
    lsl_ext = nc.declare_dram_parameter("labslice", [16, 16], i32, isOutput=False)
    wf0_ext = nc.declare_dram_parameter("wf_out0", [N // 2, CPAD], f32,
                                        isOutput=True)
    wf1_ext = nc.declare_dram_parameter("wf_out1", [N // 2, CPAD], f32,
                                        isOutput=True)
    l_ext = nc.declare_dram_parameter("l_out", [256], f32, isOutput=True)

    ar_in = nc.dram_tensor("ar_in", [128 * 48], f32)
    rs_out = nc.dram_tensor("rs_out", [16 * 48], f32)

    with ExitStack() as ctx:
        tc = ctx.enter_context(tile.TileContext(nc))
        singles = ctx.enter_context(tc.tile_pool(name="singles", bufs=1))
        xt_pool = ctx.enter_context(tc.tile_pool(name="xt", bufs=3))
        wt_pool = ctx.enter_context(tc.tile_pool(name="wt", bufs=3))
        ps_t = ctx.enter_context(tc.tile_pool(name="pst", bufs=2, space="PSUM"))
        ps_mm = ctx.enter_context(tc.tile_pool(name="psmm", bufs=6, space="PSUM"))
        wf_pool = ctx.enter_context(tc.tile_pool(name="wfout", bufs=6))
        dump_pool = ctx.enter_context(tc.tile_pool(name="edump", bufs=2))
        work = ctx.enter_context(tc.tile_pool(name="work", bufs=1))

        def bc3(ap, width):
            """[P, k] slice -> [P, k, width] broadcast (stride-0 inner)."""
            return bass.AP(tensor=ap.tensor, offset=ap.offset,
                           ap=[list(ap.ap[0]), list(ap.ap[1]), [0, width]])

        def bc2(ap, width):
            """[P, 1] column -> [P, width] broadcast."""
            return bass.AP(tensor=ap.tensor, offset=ap.offset,
                           ap=[list(ap.ap[0]), [0, width]])

        # ---- constants ----
        ident = singles.tile([128, 128], f32)
        make_identity(nc, ident[:])

        iota_i = singles.tile([128, WIN], i32)
        nc.gpsimd.iota(iota_i[:], pattern=[[1, WIN]], channel_multiplier=0)
        iota_f = singles.tile([128, WIN], f32)
        nc.vector.tensor_copy(iota_f[:], iota_i[:])
        iota_b = bass.AP(tensor=iota_f.tensor, offset=iota_f.offset,
                         ap=[list(iota_f[:].ap[0]), [0, 8], [1, WIN]])

        rmod = singles.tile([16, 128], i32)
        nc.gpsimd.iota(rmod[:].rearrange("p (a b) -> p a b", a=2),
                       pattern=[[0, 2], [320, 64]], channel_multiplier=20)

        ncl_sb = singles.tile([128, 1], i32)
        nc.sync.dma_start(out=ncl_sb[:], in_=ncl_ext[:])
        lab_pg = singles.tile([128, NG], i32)   # [p, g] = labels[g*128+p]
        nc.sync.dma_start(out=lab_pg[:],
                          in_=lab_ext[:].rearrange("(g p) -> p g", p=128))
        lab16 = singles.tile([16, 128], i32)    # [p, s] = labels[s*16+p]
        nc.sync.dma_start(out=lab16[:],
                          in_=lab_ext[:].rearrange("(s p) -> p s", p=16))
        labsl = singles.tile([16, 16], i32)
        nc.sync.dma_start(out=labsl[:], in_=lsl_ext[:])

        # ---- W shard load + transpose -> wT (4 x [128, CPAD]) ----
        wT = [singles.tile([128, CPAD], bf16, tag=f"wT{k}", name=f"wT{k}")
              for k in range(4)]
        for ct in range(CPAD // 128):
            w_t = wt_pool.tile([128, D], f32, tag="wld")
            rows = min(CSH - ct * 128, 128)
            if rows < 128:
                nc.vector.memset(w_t[:], 0.0)
            nc.sync.dma_start(out=w_t[:rows, :],
                              in_=w_ext[ct * 128: ct * 128 + rows, :])
            for k in range(4):
                pt = ps_t.tile([128, 128], f32)
                nc.tensor.transpose(pt[:], w_t[:, k * 128:(k + 1) * 128], ident[:])
                nc.vector.tensor_copy(wT[k][:, ct * 128:(ct + 1) * 128], pt[:])

        # ---- x load: row sq-norms (ACT) + raw transpose (PE) in parallel ----
        xnT = [singles.tile([128, N], bf16, tag=f"xnT{k}", name=f"xnT{k}")
               for k in range(4)]
        ss16 = singles.tile([128, NG], f32)
        for g2 in range(NG // 2):
            xp = xt_pool.tile([128, 2 * D], f32, tag="xA")
            nc.sync.dma_start(
                out=xp[:].rearrange("p (two d) -> p two d", two=2),
                in_=bass.AP(tensor=x_ext, offset=g2 * 256 * D,
                            ap=[[D, 128], [128 * D, 2], [1, D]]))
            for sub in range(2):
                g = 2 * g2 + sub
                xs = xp[:, sub * D:(sub + 1) * D]
                dmp = dump_pool.tile([128, D], f32, tag="sqdump")
                nc.scalar.activation(dmp[:], xs, Act.Square,
                                     accum_out=ss16[:, g:g + 1])
                for k in range(4):
                    pt = ps_t.tile([128, 128], f32)
                    nc.tensor.transpose(pt[:], xp[:, sub * D + k * 128:
                                                  sub * D + (k + 1) * 128],
                                        ident[:])
                    nc.vector.tensor_copy(xnT[k][:, g * 128:(g + 1) * 128], pt[:])
        inv16 = singles.tile([128, NG], f32)
        nc.scalar.activation(inv16[:], ss16[:], Act.Sqrt)
        nc.vector.tensor_scalar_max(inv16[:], inv16[:], 1e-12)
        nc.vector.reciprocal(inv16[:], inv16[:])
        sinv16 = singles.tile([128, NG], f32)
        nc.vector.tensor_scalar_mul(sinv16[:], inv16[:], S_SCALE)

        # ---- label-window bookkeeping (int math) ----
        lpg = singles.tile([128, NG], i32)
        nc.vector.tensor_tensor(lpg[:], lab_pg[:], bc2(ncl_sb[:, 0:1], NG),
                                op=Alu.add)
        bpg = singles.tile([128, NG], i32)
        nc.vector.tensor_scalar(bpg[:], lpg[:], 6, None, op0=Alu.arith_shift_right)
        nc.vector.tensor_scalar(bpg[:], bpg[:], -1, 0, op0=Alu.add, op1=Alu.max)
        nc.vector.tensor_scalar(bpg[:], bpg[:], BMAX, None, op0=Alu.min)
        apg = singles.tile([128, NG], i32)
        nc.vector.tensor_scalar_mul(apg[:], bpg[:], BLK)
        af = singles.tile([128, NG], f32)
        nc.vector.tensor_copy(af[:], apg[:])
        s0i = singles.tile([128, NG], i32)
        nc.vector.tensor_tensor(s0i[:], apg[:], lpg[:], op=Alu.subtract)
        s0f = singles.tile([128, NG], f32)
        nc.vector.tensor_copy(s0f[:], s0i[:])

        l16 = singles.tile([16, 128], i32)
        nc.vector.tensor_tensor(l16[:], lab16[:], bc2(ncl_sb[:16, 0:1], 128),
                                op=Alu.add)
        b16 = singles.tile([16, 128], i32)
        nc.vector.tensor_scalar(b16[:], l16[:], 6, None, op0=Alu.arith_shift_right)
        nc.vector.tensor_scalar(b16[:], b16[:], -1, 0, op0=Alu.add, op1=Alu.max)
        nc.vector.tensor_scalar(b16[:], b16[:], BMAX, None, op0=Alu.min)
        idx32 = singles.tile([16, 128], i32)
        nc.vector.tensor_tensor(idx32[:], b16[:], rmod[:], op=Alu.add)
        idx16 = singles.tile([128, 128], i16)
        nc.vector.memset(idx16[:], 0)
        nc.vector.tensor_copy(idx16[:16, :], idx32[:])

        # ---- normalization Z (analytic geometric sums), [16,16] slice ----
        yf = singles.tile([16, 16], f32)
        nc.vector.tensor_copy(yf[:], labsl[:])
        mL = singles.tile([16, 16], f32)
        nc.vector.tensor_scalar(mL[:], yf[:], float((C // 2) - 1), 1.0,
                                op0=Alu.min, op1=Alu.add)
        gL = singles.tile([16, 16], f32)
        nc.scalar.activation(gL[:], mL[:], Act.Exp, scale=-KDECAY)
        mR = singles.tile([16, 16], f32)
        nc.vector.tensor_scalar(mR[:], yf[:], -1.0, float(C - 1),
                                op0=Alu.mult, op1=Alu.add)
        nc.vector.tensor_scalar(mR[:], mR[:], float(C // 2), 1.0,
                                op0=Alu.min, op1=Alu.add)
        gR = singles.tile([16, 16], f32)
        nc.scalar.activation(gR[:], mR[:], Act.Exp, scale=-KDECAY)
        z_t = singles.tile([16, 16], f32)
        nc.vector.tensor_tensor(z_t[:], gL[:], gR[:], op=Alu.add)
        nc.vector.tensor_scalar(z_t[:], z_t[:], -C2, C1, op0=Alu.mult, op1=Alu.add)
        invz = singles.tile([16, 16], f32)
        nc.vector.reciprocal(invz[:], z_t[:])

        partials = singles.tile([128, 48], f32)
        esum48 = singles.tile([128, NG * 3], f32)

        HALF_ROWS = 1024
        VROWS = (HALF_ROWS - 1) * (CPAD // BLK) + BMAX + 1  # 20478

        def emit_gather(h, eng_win):
            win_t = work.tile([128, 8 * WIN], f32, tag=f"win{h}",
                              name=f"win{h}")
            gth = nc.gpsimd.dma_gather(
                out_ap=win_t[:].rearrange("p (g w) -> p g w", w=WIN),
                in_ap=bass.AP(tensor=(wf0_ext if h == 0 else wf1_ext), offset=0,
                              ap=[[BLK, VROWS], [1, WIN]]),
                idxs_ap=idx16[:, h * 64:(h + 1) * 64],
                num_idxs=HALF_ROWS,
                num_idxs_reg=HALF_ROWS,
                elem_size=WIN,
                elem_step=BLK,
            )
            return win_t, gth

        def win_early(h, eng, win_t, anchor=None):
            """d, |d|, lcol, tcl, tt2 — only needs gathered win + labels."""
            s0_b = bc3(s0f[:, h * 8:(h + 1) * 8], WIN)
            a_b = bc3(af[:, h * 8:(h + 1) * 8], WIN)
            d_t = work.tile([128, 8 * WIN], f32, tag=f"d{h}", name=f"d{h}")
            nc_e = getattr(nc, eng)
            roots = []
            roots.append(nc_e.tensor_tensor(
                d_t[:].rearrange("p (g w) -> p g w", w=WIN),
                iota_b, s0_b, op=Alu.add))
            ad_t = work.tile([128, 8 * WIN], f32, tag=f"ad{h}", name=f"ad{h}")
            nc_e.scalar_tensor_tensor(ad_t[:], d_t[:], -1.0, d_t[:],
                                      op0=Alu.mult, op1=Alu.max)
            lc_t = work.tile([128, 8 * WIN], f32, tag=f"lc{h}", name=f"lc{h}")
            roots.append(nc_e.tensor_tensor(
                lc_t[:].rearrange("p (g w) -> p g w", w=WIN),
                iota_b, a_b, op=Alu.add))
            tcl = work.tile([128, 8 * WIN], f32, tag=f"tcl{h}", name=f"tcl{h}")
            roots.append(nc_e.tensor_scalar(tcl[:], win_t[:], -CLIP, CLIP,
                                            op0=Alu.max, op1=Alu.min))
            tt2 = work.tile([128, 8 * WIN], f32, tag=f"tt2{h}", name=f"tt2{h}")
            nc_e.tensor_tensor(tt2[:], tcl[:], tcl[:], op=Alu.mult)
            if anchor is not None:
                for r in roots:
                    _add_dep_helper(r.ins, anchor.ins, False,
                                    "pin window ops late in stream")
            return d_t, ad_t, lc_t, tcl, tt2

        def win_act(h, ad_t, tt2, anchor=None):
            kern = work.tile([128, 8 * WIN], f32, tag=f"kern{h}", name=f"kern{h}")
            k_i = nc.scalar.activation(kern[:], ad_t[:], Act.Exp, scale=-KDECAY)
            s2_t = work.tile([128, 8 * WIN], f32, tag=f"s2{h}", name=f"s2{h}")
            s_i = nc.scalar.activation(s2_t[:], tt2[:], Act.Sqrt,
                                       scale=-1.0, bias=1.0)
            if anchor is not None:
                _add_dep_helper(k_i.ins, anchor.ins, False, "pin act late")
                _add_dep_helper(s_i.ins, anchor.ins, False, "pin act late")
            return kern, s2_t

        def win_late(h, eng, win_t, d_t, lc_t, tcl, kern, s2_t):
            """mask kern, u, c1; reduces go on DVE."""
            nc_e = getattr(nc, eng)
            nc_e.scalar_tensor_tensor(kern[:], lc_t[:], float(CSH), kern[:],
                                      op0=Alu.is_lt, op1=Alu.mult)
            nc_e.scalar_tensor_tensor(s2_t[:], tcl[:], A_OVER_B, s2_t[:],
                                      op0=Alu.mult, op1=Alu.add)
            nc_e.tensor_tensor(kern[:], kern[:], s2_t[:], op=Alu.mult)
            nc_e.scalar_tensor_tensor(win_t[:], d_t[:], 0.0, win_t[:],
                                      op0=Alu.is_equal, op1=Alu.mult)

        def win_reduce(h, win_t, kern):
            base = 16 + 16 * h
            nc.vector.tensor_reduce(
                partials[:, base:base + 8],
                kern[:].rearrange("p (g w) -> p g w", w=WIN),
                axis=mybir.AxisListType.X, op=Alu.add)
            nc.vector.tensor_reduce(
                partials[:, base + 8:base + 16],
                win_t[:].rearrange("p (g w) -> p g w", w=WIN),
                axis=mybir.AxisListType.X, op=Alu.add)

        # ---- main loop: f32r matmul + scaled copy + exp-sum + writeback ----
        wf_dmas = []
        copy_anchor = {}
        exp_anchor = {}
        for g in range(NG):
            wf_t = wf_pool.tile([128, CPAD], f32, tag="wf")
            for ci in range(3):
                cw, cv, c0 = CW[ci], CVALID[ci], ci * 512
                pm = ps_mm.tile([128, 512], f32, tag="mm")
                for k in range(4):
                    nc.tensor.matmul(
                        pm[:, :cw],
                        lhsT=xnT[k][:, g * 128:(g + 1) * 128],
                        rhs=wT[k][:, c0:c0 + cw],
                        start=(k == 0), stop=(k == 3))
                nc.vector.tensor_scalar_mul(wf_t[:, c0:c0 + cw], pm[:, :cw],
                                            inv16[:, g:g + 1])
                dmp = dump_pool.tile([128, 512], f32, tag="expdump")
                nc.scalar.activation(dmp[:, :cv], pm[:, :cv], Act.Exp,
                                     scale=sinv16[:, g:g + 1],
                                     accum_out=esum48[:, g * 3 + ci:g * 3 + ci + 1])
            copy_anchor[g] = cp_i
            exp_anchor[g] = ex_i
            wf_half = wf0_ext if g < 8 else wf1_ext
            r0 = (g % 8) * 128
            dma = nc.sync.dma_start(out=wf_half[r0:r0 + 128, :], in_=wf_t[:])
            wf_dmas.append(dma)

            if g == 7:
                win0, gth0 = emit_gather(0, "gpsimd")
                for dma in wf_dmas[:8]:
                    _add_dep_helper(gth0.ins, dma.ins, True, "gather0 after wf g0-7")
            if g == 12:
                w0_early = win_early(0, "vector", win0, anchor=copy_anchor[11])
            if g == 13:
                d0, ad0, lc0, tcl0, tt20 = w0_early
                kern0, s20 = win_act(0, ad0, tt20, anchor=exp_anchor[12])
            if g == 14:
                win_late(0, "vector", win0, d0, lc0, tcl0, kern0, s20)

        win_reduce(0, win0, kern0)
        # exp-sum reduction into partials (needs all 48 accums)
        nc.vector.tensor_reduce(
            partials[:, 0:NG],
            esum48[:].rearrange("p (g c) -> p g c", c=3),
            axis=mybir.AxisListType.X, op=Alu.add)
        # RS_a: esum + first-half term1/wfy — overlaps gather1/window1
        nc.sync.dma_start(
            out=ara_in[:].rearrange("(p f) -> p f", p=128),
            in_=partials[:, 0:32])
        nc.gpsimd.collective_compute(
            "ReduceScatter", mybir.AluOpType.add,
            replica_groups=[list(range(NCORES))],
            ins=[ara_in[:]], outs=[rsa_out[:]])
        red_a = singles.tile([16, 32], f32)
        nc.sync.dma_start(out=red_a[:],
                          in_=rsa_out[:].rearrange("(p f) -> p f", p=16))

        # ---- second-half window phase (post-loop, DVE has idle time) ----
        win1, gth1 = emit_gather(1, "vector")
        for dma in wf_dmas[8:]:
            _add_dep_helper(gth1.ins, dma.ins, True, "gather1 after wf g8-15")
        d1, ad1, lc1, tcl1, tt21 = win_early(1, "vector", win1)
        kern1, s21 = win_act(1, ad1, tt21)
        win_late(1, "vector", win1, d1, lc1, tcl1, kern1, s21)
        win_reduce(1, win1, kern1)

        # RS_b: second-half term1/wfy
        nc.sync.dma_start(
            out=arb_in[:].rearrange("(p f) -> p f", p=128),
            in_=partials[:, 32:48])
        nc.gpsimd.collective_compute(
            "ReduceScatter", mybir.AluOpType.add,
            replica_groups=[list(range(NCORES))],
            ins=[arb_in[:]], outs=[rsb_out[:]])
        red_b = singles.tile([16, 16], f32)
        nc.sync.dma_start(out=red_b[:],
                          in_=rsb_out[:].rearrange("(p f) -> p f", p=16))
        esumT = red_a[:, 0:16]
        t1T = singles.tile([16, 16], f32)
        nc.vector.tensor_copy(t1T[:, 0:8], red_a[:, 16:24])
        nc.vector.tensor_copy(t1T[:, 8:16], red_b[:, 0:8])
        wfyT = singles.tile([16, 16], f32)
        nc.vector.tensor_copy(wfyT[:, 0:8], red_a[:, 24:32])
        nc.vector.tensor_copy(wfyT[:, 8:16], red_b[:, 8:16])
        t1T = t1T[:]
        wfyT = wfyT[:]

        # ---- final per-row loss slice ----
        tcy = singles.tile([16, 16], f32)
        nc.vector.tensor_scalar(tcy[:], wfyT, -CLIP, CLIP, op0=Alu.max, op1=Alu.min)
        ty2 = singles.tile([16, 16], f32)
        nc.vector.tensor_tensor(ty2[:], tcy[:], tcy[:], op=Alu.mult)
        s2y = singles.tile([16, 16], f32)
        nc.scalar.activation(s2y[:], ty2[:], Act.Sqrt, scale=-1.0, bias=1.0)
        tyA = singles.tile([16, 16], f32)
        nc.vector.tensor_scalar_mul(tyA[:], tcy[:], A_COS)
        numy = singles.tile([16, 16], f32)
        nc.vector.scalar_tensor_tensor(numy[:], s2y[:], B_SIN, tyA[:],
                                       op0=Alu.mult, op1=Alu.add)
        eny = singles.tile([16, 16], f32)
        nc.scalar.activation(eny[:], numy[:], Act.Exp)
        ey = singles.tile([16, 16], f32)
        nc.scalar.activation(ey[:], wfyT, Act.Exp, scale=S_SCALE)
        den = singles.tile([16, 16], f32)
        nc.vector.tensor_tensor(den[:], eny[:], esumT, op=Alu.add)
        nc.vector.tensor_tensor(den[:], den[:], ey[:], op=Alu.subtract)
        lden = singles.tile([16, 16], f32)
        nc.scalar.activation(lden[:], den[:], Act.Ln)
        q1 = singles.tile([16, 16], f32)
        nc.vector.tensor_tensor(q1[:], t1T, invz[:], op=Alu.mult)
        l_t = singles.tile([16, 16], f32)
        nc.vector.scalar_tensor_tensor(l_t[:], q1[:], B_SIN, lden[:],
                                       op0=Alu.mult, op1=Alu.subtract)
        nc.sync.dma_start(out=l_ext[:].rearrange("(pp g) -> pp g", g=16),
                          in_=l_t[:])

    nc.finalize()
    return nc


def _get_nc():
    if "nc" not in _CACHE:
        _CACHE["nc"] = _build()
    return _CACHE["nc"]


def make_in_maps(x, labels, W):
    x = np.ascontiguousarray(x, dtype=np.float32)
    W = np.ascontiguousarray(W, dtype=np.float32)
    labels = np.ascontiguousarray(labels, dtype=np.int32)
    lab2d = labels.reshape(NG, 128)  # [g, p]
    in_maps = []
    for i in range(NCORES):
        # labslice[pp, g] = labels[g*128 + 16*i + pp]
        lsl = np.ascontiguousarray(lab2d[:, 16 * i:16 * (i + 1)].T)
        in_maps.append({
            "xsl": np.ascontiguousarray(x[i * 256:(i + 1) * 256]),
            "w": np.ascontiguousarray(W[i * CSH:(i + 1) * CSH]),
            "labels": labels,
            "negclo": np.full((128, 1), -i * CSH, dtype=np.int32),
            "labslice": lsl,
        })
    return in_maps


def assemble(outs):
    """outs: per-core dicts with 'wf_out0'/'wf_out1' [N/2, CPAD], 'l_out' [256]."""
    wf = np.concatenate(
        [np.concatenate([outs[i]["wf_out0"], outs[i]["wf_out1"]], axis=0)[:, :CSH]
         for i in range(NCORES)], axis=1)
    l2d = np.zeros((NG, 128), dtype=np.float32)  # [g, p]
    for i in range(NCORES):
        l2d[:, 16 * i:16 * (i + 1)] = outs[i]["l_out"].reshape(16, 16).T
    loss = np.float32(-np.mean(l2d))
    return wf, loss


def kernel(x, labels, W):
    from concourse.bass_utils import run_bass_kernel_spmd

    nc = _get_nc()
    in_maps = make_in_maps(x, labels, W)
    res = run_bass_kernel_spmd(nc, in_maps, core_ids=list(range(NCORES)))
    return assemble(res.results)


# revision 24
# speedup vs baseline: 1.6739x; 1.0626x over previous
"""Trainium2 Bass kernel for AngularPenaltySMLossWithSoftLabel.

Strategy: tensor-parallel over out_features C=10000 across 8 cores
(1250 columns each). Each core:
  - transposes its W shard and the full (raw) x on the PE,
  - computes wf[:, c_lo:c_hi] = x @ W_shard.T in float32r, scaling rows
    by 1/||x_n|| in the epilogue (copy + fused exp-sum accumulate),
  - gathers a 192-wide window around each row's label (clipped to its
    shard) from its wf shard via indirect DMA and computes the
    soft-label kernel * arcface-numerator contribution (the
    exp(-0.2*d) kernel is < 3e-6 outside +-64),
  - ReduceScatters [exp_sum, term1, wf_y] (3 x 2048 f32) across cores,
  - computes the final per-row loss slice L (256 rows/core) on device.
Host glue: concatenate wf shards, slice padding, loss = -mean(L).
"""

import math
import numpy as np

N, D, C = 2048, 512, 10000
NCORES = 8
CSH = C // NCORES          # 1250 columns per core
CPAD = 1280                # padded row stride of the wf output (256B multiple)
S_SCALE = 64.0
MARGIN = 0.5
EPS = 1e-7
KDECAY = 0.2

A_COS = S_SCALE * math.cos(MARGIN)            # 56.1652...
B_SIN = -S_SCALE * math.sin(MARGIN)           # -30.6832...
A_OVER_B = math.cos(MARGIN) / -math.sin(MARGIN)  # -1.8304877...
CLIP = 1.0 - EPS
G = math.exp(-KDECAY)
C2 = 1.0 / (1.0 - G)                          # 5.51667
C1 = 1.0 + 2.0 * G * C2                       # 10.03331

WIN = 192          # gather window width (64-aligned, covers label +-64)
BLK = 64           # gather stride granularity (256B in f32)
BMAX = (CPAD - WIN) // BLK   # 17: max window block start
NG = N // 128      # 16 row-chunks of 128
CW = [512, 512, 256]         # c-chunk widths (local columns)
CVALID = [512, 512, CSH - 1024]  # valid (non-ghost) widths

_CACHE = {}


def _build():
    import concourse.bass as bass
    import concourse.mybir as mybir
    from concourse import bacc, tile
    from concourse.masks import make_identity
    from concourse.bass import _add_dep_helper
    from contextlib import ExitStack

    f32 = mybir.dt.float32
    f32r = mybir.dt.float32r
    bf16 = mybir.dt.bfloat16
    i32 = mybir.dt.int32
    i16 = mybir.dt.int16
    Alu = mybir.AluOpType
    Act = mybir.ActivationFunctionType

    nc = bacc.Bacc("TRN2", num_devices=NCORES)

    x_ext = nc.declare_dram_parameter("x", [N, D], f32, isOutput=False)
    w_ext = nc.declare_dram_parameter("w", [CSH, D], f32, isOutput=False)
    lab_ext = nc.declare_dram_parameter("labels", [N], i32, isOutput=False)
    ncl_ext = nc.declare_dram_parameter("negclo", [128, 1], i32, isOutput=False)
    lsl_ext = nc.declare_dram_parameter("labslice", [16, 16], i32, isOutput=False)
    wf0_ext = nc.declare_dram_parameter("wf_out0", [N // 2, CPAD], f32,
                                        isOutput=True)
    wf1_ext = nc.declare_dram_parameter("wf_out1", [N // 2, CPAD], f32,
                                        isOutput=True)
    l_ext = nc.declare_dram_parameter("l_out", [256], f32, isOutput=True)

    ar_in = nc.dram_tensor("ar_in", [128 * 48], f32)
    rs_out = nc.dram_tensor("rs_out", [16 * 48], f32)

    with ExitStack() as ctx:
        tc = ctx.enter_context(tile.TileContext(nc))
        singles = ctx.enter_context(tc.tile_pool(name="singles", bufs=1))
        xt_pool = ctx.enter_context(tc.tile_pool(name="xt", bufs=3))
        wt_pool = ctx.enter_context(tc.tile_pool(name="wt", bufs=3))
        ps_t = ctx.enter_context(tc.tile_pool(name="pst", bufs=2, space="PSUM"))
        ps_mm = ctx.enter_context(tc.tile_pool(name="psmm", bufs=6, space="PSUM"))
        wf_pool = ctx.enter_context(tc.tile_pool(name="wfout", bufs=6))
        dump_pool = ctx.enter_context(tc.tile_pool(name="edump", bufs=2))
        work = ctx.enter_context(tc.tile_pool(name="work", bufs=1))

        def bc3(ap, width):
            """[P, k] slice -> [P, k, width] broadcast (stride-0 inner)."""
            return bass.AP(tensor=ap.tensor, offset=ap.offset,
                           ap=[list(ap.ap[0]), list(ap.ap[1]), [0, width]])

        def bc2(ap, width):
            """[P, 1] column -> [P, width] broadcast."""
            return bass.AP(tensor=ap.tensor, offset=ap.offset,
                           ap=[list(ap.ap[0]), [0, width]])

        # ---- constants ----
        ident = singles.tile([128, 128], f32)
        make_identity(nc, ident[:])

        iota_i = singles.tile([128, WIN], i32)
        nc.gpsimd.iota(iota_i[:], pattern=[[1, WIN]], channel_multiplier=0)
        iota_f = singles.tile([128, WIN], f32)
        nc.vector.tensor_copy(iota_f[:], iota_i[:])
        iota_b = bass.AP(tensor=iota_f.tensor, offset=iota_f.offset,
                         ap=[list(iota_f[:].ap[0]), [0, 8], [1, WIN]])

        rmod = singles.tile([16, 128], i32)
        nc.gpsimd.iota(rmod[:].rearrange("p (a b) -> p a b", a=2),
                       pattern=[[0, 2], [320, 64]], channel_multiplier=20)

        ncl_sb = singles.tile([128, 1], i32)
        nc.sync.dma_start(out=ncl_sb[:], in_=ncl_ext[:])
        lab_pg = singles.tile([128, NG], i32)   # [p, g] = labels[g*128+p]
        nc.sync.dma_start(out=lab_pg[:],
                          in_=lab_ext[:].rearrange("(g p) -> p g", p=128))
        lab16 = singles.tile([16, 128], i32)    # [p, s] = labels[s*16+p]
        nc.sync.dma_start(out=lab16[:],
                          in_=lab_ext[:].rearrange("(s p) -> p s", p=16))
        labsl = singles.tile([16, 16], i32)
        nc.sync.dma_start(out=labsl[:], in_=lsl_ext[:])

        # ---- W shard load + transpose -> wT (4 x [128, CPAD]) ----
        wT = [singles.tile([128, CPAD], bf16, tag=f"wT{k}", name=f"wT{k}")
              for k in range(4)]
        for ct in range(CPAD // 128):
            w_t = wt_pool.tile([128, D], f32, tag="wld")
            rows = min(CSH - ct * 128, 128)
            if rows < 128:
                nc.vector.memset(w_t[:], 0.0)
            nc.sync.dma_start(out=w_t[:rows, :],
                              in_=w_ext[ct * 128: ct * 128 + rows, :])
            for k in range(4):
                pt = ps_t.tile([128, 128], f32)
                nc.tensor.transpose(pt[:], w_t[:, k * 128:(k + 1) * 128], ident[:])
                nc.vector.tensor_copy(wT[k][:, ct * 128:(ct + 1) * 128], pt[:])

        # ---- x load: row sq-norms (ACT) + raw transpose (PE) in parallel ----
        xnT = [singles.tile([128, N], bf16, tag=f"xnT{k}", name=f"xnT{k}")
               for k in range(4)]
        ss16 = singles.tile([128, NG], f32)
        for g2 in range(NG // 2):
            xp = xt_pool.tile([128, 2 * D], f32, tag="xA")
            nc.sync.dma_start(
                out=xp[:].rearrange("p (two d) -> p two d", two=2),
                in_=bass.AP(tensor=x_ext, offset=g2 * 256 * D,
                            ap=[[D, 128], [128 * D, 2], [1, D]]))
            for sub in range(2):
                g = 2 * g2 + sub
                xs = xp[:, sub * D:(sub + 1) * D]
                dmp = dump_pool.tile([128, D], f32, tag="sqdump")
                nc.scalar.activation(dmp[:], xs, Act.Square,
                                     accum_out=ss16[:, g:g + 1])
                for k in range(4):
                    pt = ps_t.tile([128, 128], f32)
                    nc.tensor.transpose(pt[:], xp[:, sub * D + k * 128:
                                                  sub * D + (k + 1) * 128],
                                        ident[:])
                    nc.vector.tensor_copy(xnT[k][:, g * 128:(g + 1) * 128], pt[:])
        inv16 = singles.tile([128, NG], f32)
        nc.scalar.activation(inv16[:], ss16[:], Act.Sqrt)
        nc.vector.tensor_scalar_max(inv16[:], inv16[:], 1e-12)
        nc.vector.reciprocal(inv16[:], inv16[:])
        sinv16 = singles.tile([128, NG], f32)
        nc.vector.tensor_scalar_mul(sinv16[:], inv16[:], S_SCALE)

        # ---- label-window bookkeeping (int math) ----
        lpg = singles.tile([128, NG], i32)
        nc.vector.tensor_tensor(lpg[:], lab_pg[:], bc2(ncl_sb[:, 0:1], NG),
                                op=Alu.add)
        bpg = singles.tile([128, NG], i32)
        nc.vector.tensor_scalar(bpg[:], lpg[:], 6, None, op0=Alu.arith_shift_right)
        nc.vector.tensor_scalar(bpg[:], bpg[:], -1, 0, op0=Alu.add, op1=Alu.max)
        nc.vector.tensor_scalar(bpg[:], bpg[:], BMAX, None, op0=Alu.min)
        apg = singles.tile([128, NG], i32)
        nc.vector.tensor_scalar_mul(apg[:], bpg[:], BLK)
        af = singles.tile([128, NG], f32)
        nc.vector.tensor_copy(af[:], apg[:])
        s0i = singles.tile([128, NG], i32)
        nc.vector.tensor_tensor(s0i[:], apg[:], lpg[:], op=Alu.subtract)
        s0f = singles.tile([128, NG], f32)
        nc.vector.tensor_copy(s0f[:], s0i[:])

        l16 = singles.tile([16, 128], i32)
        nc.vector.tensor_tensor(l16[:], lab16[:], bc2(ncl_sb[:16, 0:1], 128),
                                op=Alu.add)
        b16 = singles.tile([16, 128], i32)
        nc.vector.tensor_scalar(b16[:], l16[:], 6, None, op0=Alu.arith_shift_right)
        nc.vector.tensor_scalar(b16[:], b16[:], -1, 0, op0=Alu.add, op1=Alu.max)
        nc.vector.tensor_scalar(b16[:], b16[:], BMAX, None, op0=Alu.min)
        idx32 = singles.tile([16, 128], i32)
        nc.vector.tensor_tensor(idx32[:], b16[:], rmod[:], op=Alu.add)
        idx16 = singles.tile([128, 128], i16)
        nc.vector.memset(idx16[:], 0)
        nc.vector.tensor_copy(idx16[:16, :], idx32[:])

        # ---- normalization Z (analytic geometric sums), [16,16] slice ----
        yf = singles.tile([16, 16], f32)
        nc.vector.tensor_copy(yf[:], labsl[:])
        mL = singles.tile([16, 16], f32)
        nc.vector.tensor_scalar(mL[:], yf[:], float((C // 2) - 1), 1.0,
                                op0=Alu.min, op1=Alu.add)
        gL = singles.tile([16, 16], f32)
        nc.scalar.activation(gL[:], mL[:], Act.Exp, scale=-KDECAY)
        mR = singles.tile([16, 16], f32)
        nc.vector.tensor_scalar(mR[:], yf[:], -1.0, float(C - 1),
                                op0=Alu.mult, op1=Alu.add)
        nc.vector.tensor_scalar(mR[:], mR[:], float(C // 2), 1.0,
                                op0=Alu.min, op1=Alu.add)
        gR = singles.tile([16, 16], f32)
        nc.scalar.activation(gR[:], mR[:], Act.Exp, scale=-KDECAY)
        z_t = singles.tile([16, 16], f32)
        nc.vector.tensor_tensor(z_t[:], gL[:], gR[:], op=Alu.add)
        nc.vector.tensor_scalar(z_t[:], z_t[:], -C2, C1, op0=Alu.mult, op1=Alu.add)
        invz = singles.tile([16, 16], f32)
        nc.vector.reciprocal(invz[:], z_t[:])

        partials = singles.tile([128, 48], f32)
        esum48 = singles.tile([128, NG * 3], f32)

        HALF_ROWS = 1024
        VROWS = (HALF_ROWS - 1) * (CPAD // BLK) + BMAX + 1  # 20478

        def emit_gather(h, eng_win):
            win_t = work.tile([128, 8 * WIN], f32, tag=f"win{h}",
                              name=f"win{h}")
            gth = nc.gpsimd.dma_gather(
                out_ap=win_t[:].rearrange("p (g w) -> p g w", w=WIN),
                in_ap=bass.AP(tensor=(wf0_ext if h == 0 else wf1_ext), offset=0,
                              ap=[[BLK, VROWS], [1, WIN]]),
                idxs_ap=idx16[:, h * 64:(h + 1) * 64],
                num_idxs=HALF_ROWS,
                num_idxs_reg=HALF_ROWS,
                elem_size=WIN,
                elem_step=BLK,
            )
            return win_t, gth

        def win_early(h, eng, win_t, anchor=None):
            """d, |d|, lcol, tcl, tt2 — only needs gathered win + labels."""
            s0_b = bc3(s0f[:, h * 8:(h + 1) * 8], WIN)
            a_b = bc3(af[:, h * 8:(h + 1) * 8], WIN)
            d_t = work.tile([128, 8 * WIN], f32, tag=f"d{h}", name=f"d{h}")
            nc_e = getattr(nc, eng)
            roots = []
            roots.append(nc_e.tensor_tensor(
                d_t[:].rearrange("p (g w) -> p g w", w=WIN),
                iota_b, s0_b, op=Alu.add))
            ad_t = work.tile([128, 8 * WIN], f32, tag=f"ad{h}", name=f"ad{h}")
            nc_e.scalar_tensor_tensor(ad_t[:], d_t[:], -1.0, d_t[:],
                                      op0=Alu.mult, op1=Alu.max)
            lc_t = work.tile([128, 8 * WIN], f32, tag=f"lc{h}", name=f"lc{h}")
            roots.append(nc_e.tensor_tensor(
                lc_t[:].rearrange("p (g w) -> p g w", w=WIN),
                iota_b, a_b, op=Alu.add))
            tcl = work.tile([128, 8 * WIN], f32, tag=f"tcl{h}", name=f"tcl{h}")
            roots.append(nc_e.tensor_scalar(tcl[:], win_t[:], -CLIP, CLIP,
                                            op0=Alu.max, op1=Alu.min))
            tt2 = work.tile([128, 8 * WIN], f32, tag=f"tt2{h}", name=f"tt2{h}")
            nc_e.tensor_tensor(tt2[:], tcl[:], tcl[:], op=Alu.mult)
            if anchor is not None:
                for r in roots:
                    _add_dep_helper(r.ins, anchor.ins, False,
                                    "pin window ops late in stream")
            return d_t, ad_t, lc_t, tcl, tt2

        def win_act(h, ad_t, tt2, anchor=None):
            kern = work.tile([128, 8 * WIN], f32, tag=f"kern{h}", name=f"kern{h}")
            k_i = nc.scalar.activation(kern[:], ad_t[:], Act.Exp, scale=-KDECAY)
            s2_t = work.tile([128, 8 * WIN], f32, tag=f"s2{h}", name=f"s2{h}")
            s_i = nc.scalar.activation(s2_t[:], tt2[:], Act.Sqrt,
                                       scale=-1.0, bias=1.0)
            if anchor is not None:
                _add_dep_helper(k_i.ins, anchor.ins, False, "pin act late")
                _add_dep_helper(s_i.ins, anchor.ins, False, "pin act late")
            return kern, s2_t

        def win_late(h, eng, win_t, d_t, lc_t, tcl, kern, s2_t):
            """mask kern, u, c1; reduces go on DVE."""
            nc_e = getattr(nc, eng)
            nc_e.scalar_tensor_tensor(kern[:], lc_t[:], float(CSH), kern[:],
                                      op0=Alu.is_lt, op1=Alu.mult)
            nc_e.scalar_tensor_tensor(s2_t[:], tcl[:], A_OVER_B, s2_t[:],
                                      op0=Alu.mult, op1=Alu.add)
            nc_e.tensor_tensor(kern[:], kern[:], s2_t[:], op=Alu.mult)
            nc_e.scalar_tensor_tensor(win_t[:], d_t[:], 0.0, win_t[:],
                                      op0=Alu.is_equal, op1=Alu.mult)

        def win_reduce(h, win_t, kern):
            base = 16 + 16 * h
            nc.vector.tensor_reduce(
                partials[:, base:base + 8],
                kern[:].rearrange("p (g w) -> p g w", w=WIN),
                axis=mybir.AxisListType.X, op=Alu.add)
            nc.vector.tensor_reduce(
                partials[:, base + 8:base + 16],
                win_t[:].rearrange("p (g w) -> p g w", w=WIN),
                axis=mybir.AxisListType.X, op=Alu.add)

        # ---- main loop: f32r matmul + scaled copy + exp-sum + writeback ----
        wf_dmas = []
        copy_anchor = {}
        exp_anchor = {}
        for g in range(NG):
            wf_t = wf_pool.tile([128, CPAD], f32, tag="wf")
            for ci in range(3):
                cw, cv, c0 = CW[ci], CVALID[ci], ci * 512
                pm = ps_mm.tile([128, 512], f32, tag="mm")
                for k in range(4):
                    nc.tensor.matmul(
                        pm[:, :cw],
                        lhsT=xnT[k][:, g * 128:(g + 1) * 128],
                        rhs=wT[k][:, c0:c0 + cw],
                        start=(k == 0), stop=(k == 3))
                nc.vector.tensor_scalar_mul(wf_t[:, c0:c0 + cw], pm[:, :cw],
                                            inv16[:, g:g + 1])
                dmp = dump_pool.tile([128, 512], f32, tag="expdump")
                nc.scalar.activation(dmp[:, :cv], pm[:, :cv], Act.Exp,
                                     scale=sinv16[:, g:g + 1],
                                     accum_out=esum48[:, g * 3 + ci:g * 3 + ci + 1])
            copy_anchor[g] = cp_i
            exp_anchor[g] = ex_i
            wf_half = wf0_ext if g < 8 else wf1_ext
            r0 = (g % 8) * 128
            dma = nc.sync.dma_start(out=wf_half[r0:r0 + 128, :], in_=wf_t[:])
            wf_dmas.append(dma)

            if g == 7:
                win0, gth0 = emit_gather(0, "gpsimd")
                for dma in wf_dmas[:8]:
                    _add_dep_helper(gth0.ins, dma.ins, True, "gather0 after wf g0-7")
            if g == 12:
                w0_early = win_early(0, "vector", win0, anchor=copy_anchor[11])
            if g == 13:
                d0, ad0, lc0, tcl0, tt20 = w0_early
                kern0, s20 = win_act(0, ad0, tt20, anchor=exp_anchor[12])
            if g == 14:
                win_late(0, "vector", win0, d0, lc0, tcl0, kern0, s20)

        # gather1 launch first (long Q7 launch pole)
        win1, gth1 = emit_gather(1, "vector")
        for dma in wf_dmas[8:]:
            _add_dep_helper(gth1.ins, dma.ins, True, "gather1 after wf g8-15")

        # RS_a: exp-sums only — ready at loop end, overlaps both windows
        nc.vector.tensor_reduce(
            partials[:, 0:NG],
            esum48[:].rearrange("p (g c) -> p g c", c=3),
            axis=mybir.AxisListType.X, op=Alu.add)
        nc.sync.dma_start(
            out=ara_in[:].rearrange("(p f) -> p f", p=128),
            in_=partials[:, 0:16])
        nc.gpsimd.collective_compute(
            "ReduceScatter", mybir.AluOpType.add,
            replica_groups=[list(range(NCORES))],
            ins=[ara_in[:]], outs=[rsa_out[:]])
        red_a = singles.tile([16, 16], f32)
        nc.sync.dma_start(out=red_a[:],
                          in_=rsa_out[:].rearrange("(p f) -> p f", p=16))

        win_reduce(0, win0, kern0)
        # ---- second-half window phase (post-loop, DVE has idle time) ----
        d1, ad1, lc1, tcl1, tt21 = win_early(1, "vector", win1)
        kern1, s21 = win_act(1, ad1, tt21)
        win_late(1, "vector", win1, d1, lc1, tcl1, kern1, s21)
        win_reduce(1, win1, kern1)

        # RS_b: all window terms (t1/wfy for both halves)
        nc.sync.dma_start(
            out=arb_in[:].rearrange("(p f) -> p f", p=128),
            in_=partials[:, 16:48])
        nc.gpsimd.collective_compute(
            "ReduceScatter", mybir.AluOpType.add,
            replica_groups=[list(range(NCORES))],
            ins=[arb_in[:]], outs=[rsb_out[:]])
        red_b = singles.tile([16, 32], f32)
        nc.sync.dma_start(out=red_b[:],
                          in_=rsb_out[:].rearrange("(p f) -> p f", p=16))
        esumT = red_a[:, 0:16]
        t1T = singles.tile([16, 16], f32)
        nc.vector.tensor_copy(t1T[:, 0:8], red_b[:, 0:8])
        nc.vector.tensor_copy(t1T[:, 8:16], red_b[:, 16:24])
        wfyT = singles.tile([16, 16], f32)
        nc.vector.tensor_copy(wfyT[:, 0:8], red_b[:, 8:16])
        nc.vector.tensor_copy(wfyT[:, 8:16], red_b[:, 24:32])
        t1T = t1T[:]
        wfyT = wfyT[:]

        # ---- final per-row loss slice ----
        tcy = singles.tile([16, 16], f32)
        nc.vector.tensor_scalar(tcy[:], wfyT, -CLIP, CLIP, op0=Alu.max, op1=Alu.min)
        ty2 = singles.tile([16, 16], f32)
        nc.vector.tensor_tensor(ty2[:], tcy[:], tcy[:], op=Alu.mult)
        s2y = singles.tile([16, 16], f32)
        nc.scalar.activation(s2y[:], ty2[:], Act.Sqrt, scale=-1.0, bias=1.0)
        tyA = singles.tile([16, 16], f32)
        nc.vector.tensor_scalar_mul(tyA[:], tcy[:], A_COS)
        numy = singles.tile([16, 16], f32)
        nc.vector.scalar_tensor_tensor(numy[:], s2y[:], B_SIN, tyA[:],
                                       op0=Alu.mult, op1=Alu.add)
        eny = singles.tile([16, 16], f32)
        nc.scalar.activation(eny[:], numy[:], Act.Exp)
        ey = singles.tile([16, 16], f32)
        nc.scalar.activation(ey[:], wfyT, Act.Exp, scale=S_SCALE)
        den = singles.tile([16, 16], f32)
        nc.vector.tensor_tensor(den[:], eny[:], esumT, op=Alu.add)
        nc.vector.tensor_tensor(den[:], den[:], ey[:], op=Alu.subtract)
        lden = singles.tile([16, 16], f32)
        nc.scalar.activation(lden[:], den[:], Act.Ln)
        q1 = singles.tile([16, 16], f32)
        nc.vector.tensor_tensor(q1[:], t1T, invz[:], op=Alu.mult)
        l_t = singles.tile([16, 16], f32)
        nc.vector.scalar_tensor_tensor(l_t[:], q1[:], B_SIN, lden[:],
                                       op0=Alu.mult, op1=Alu.subtract)
        nc.sync.dma_start(out=l_ext[:].rearrange("(pp g) -> pp g", g=16),
                          in_=l_t[:])

    nc.finalize()
    return nc


def _get_nc():
    if "nc" not in _CACHE:
        _CACHE["nc"] = _build()
    return _CACHE["nc"]


def make_in_maps(x, labels, W):
    x = np.ascontiguousarray(x, dtype=np.float32)
    W = np.ascontiguousarray(W, dtype=np.float32)
    labels = np.ascontiguousarray(labels, dtype=np.int32)
    lab2d = labels.reshape(NG, 128)  # [g, p]
    in_maps = []
    for i in range(NCORES):
        # labslice[pp, g] = labels[g*128 + 16*i + pp]
        lsl = np.ascontiguousarray(lab2d[:, 16 * i:16 * (i + 1)].T)
        in_maps.append({
            "xsl": np.ascontiguousarray(x[i * 256:(i + 1) * 256]),
            "w": np.ascontiguousarray(W[i * CSH:(i + 1) * CSH]),
            "labels": labels,
            "negclo": np.full((128, 1), -i * CSH, dtype=np.int32),
            "labslice": lsl,
        })
    return in_maps


def assemble(outs):
    """outs: per-core dicts with 'wf_out0'/'wf_out1' [N/2, CPAD], 'l_out' [256]."""
    wf = np.concatenate(
        [np.concatenate([outs[i]["wf_out0"], outs[i]["wf_out1"]], axis=0)[:, :CSH]
         for i in range(NCORES)], axis=1)
    l2d = np.zeros((NG, 128), dtype=np.float32)  # [g, p]
    for i in range(NCORES):
        l2d[:, 16 * i:16 * (i + 1)] = outs[i]["l_out"].reshape(16, 16).T
    loss = np.float32(-np.mean(l2d))
    return wf, loss


def kernel(x, labels, W):
    from concourse.bass_utils import run_bass_kernel_spmd

    nc = _get_nc()
    in_maps = make_in_maps(x, labels, W)
    res = run_bass_kernel_spmd(nc, in_maps, core_ids=list(range(NCORES)))
    return assemble(res.results)


# revision 25
# speedup vs baseline: 1.6851x; 1.0067x over previous
"""Trainium2 Bass kernel for AngularPenaltySMLossWithSoftLabel.

Strategy: tensor-parallel over out_features C=10000 across 8 cores
(1250 columns each). Each core:
  - transposes its W shard and the full (raw) x on the PE,
  - computes wf[:, c_lo:c_hi] = x @ W_shard.T in float32r, scaling rows
    by 1/||x_n|| in the epilogue (copy + fused exp-sum accumulate),
  - gathers a 192-wide window around each row's label (clipped to its
    shard) from its wf shard via indirect DMA and computes the
    soft-label kernel * arcface-numerator contribution (the
    exp(-0.2*d) kernel is < 3e-6 outside +-64),
  - ReduceScatters [exp_sum, term1, wf_y] (3 x 2048 f32) across cores,
  - computes the final per-row loss slice L (256 rows/core) on device.
Host glue: concatenate wf shards, slice padding, loss = -mean(L).
"""

import math
import numpy as np

N, D, C = 2048, 512, 10000
NCORES = 8
CSH = C // NCORES          # 1250 columns per core
CPAD = 1280                # padded row stride of the wf output (256B multiple)
S_SCALE = 64.0
MARGIN = 0.5
EPS = 1e-7
KDECAY = 0.2

A_COS = S_SCALE * math.cos(MARGIN)            # 56.1652...
B_SIN = -S_SCALE * math.sin(MARGIN)           # -30.6832...
A_OVER_B = math.cos(MARGIN) / -math.sin(MARGIN)  # -1.8304877...
CLIP = 1.0 - EPS
G = math.exp(-KDECAY)
C2 = 1.0 / (1.0 - G)                          # 5.51667
C1 = 1.0 + 2.0 * G * C2                       # 10.03331

WIN = 192          # gather window width (64-aligned, covers label +-64)
BLK = 64           # gather stride granularity (256B in f32)
BMAX = (CPAD - WIN) // BLK   # 17: max window block start
NG = N // 128      # 16 row-chunks of 128
CW = [512, 512, 256]         # c-chunk widths (local columns)
CVALID = [512, 512, CSH - 1024]  # valid (non-ghost) widths

_CACHE = {}


def _build():
    import concourse.bass as bass
    import concourse.mybir as mybir
    from concourse import bacc, tile
    from concourse.masks import make_identity
    from concourse.bass import _add_dep_helper
    from contextlib import ExitStack

    f32 = mybir.dt.float32
    f32r = mybir.dt.float32r
    bf16 = mybir.dt.bfloat16
    i32 = mybir.dt.int32
    i16 = mybir.dt.int16
    Alu = mybir.AluOpType
    Act = mybir.ActivationFunctionType

    nc = bacc.Bacc("TRN2", num_devices=NCORES)

    x_ext = nc.declare_dram_parameter("x", [N, D], f32, isOutput=False)
    w_ext = nc.declare_dram_parameter("w", [CSH, D], f32, isOutput=False)
    lab_ext = nc.declare_dram_parameter("labels", [N], i32, isOutput=False)
    ncl_ext = nc.declare_dram_parameter("negclo", [128, 1], i32, isOutput=False)
    lsl_ext = nc.declare_dram_parameter("labslice", [16, 16], i32, isOutput=False)
    wf0_ext = nc.declare_dram_parameter("wf_out0", [N // 2, CPAD], f32,
                                        isOutput=True)
    wf1_ext = nc.declare_dram_parameter("wf_out1", [N // 2, CPAD], f32,
                                        isOutput=True)
    l_ext = nc.declare_dram_parameter("l_out", [256], f32, isOutput=True)

    ar_in = nc.dram_tensor("ar_in", [128 * 48], f32)
    rs_out = nc.dram_tensor("rs_out", [16 * 48], f32)

    with ExitStack() as ctx:
        tc = ctx.enter_context(tile.TileContext(nc))
        singles = ctx.enter_context(tc.tile_pool(name="singles", bufs=1))
        xt_pool = ctx.enter_context(tc.tile_pool(name="xt", bufs=3))
        wt_pool = ctx.enter_context(tc.tile_pool(name="wt", bufs=3))
        ps_t = ctx.enter_context(tc.tile_pool(name="pst", bufs=2, space="PSUM"))
        ps_mm = ctx.enter_context(tc.tile_pool(name="psmm", bufs=6, space="PSUM"))
        wf_pool = ctx.enter_context(tc.tile_pool(name="wfout", bufs=6))
        dump_pool = ctx.enter_context(tc.tile_pool(name="edump", bufs=2))
        work = ctx.enter_context(tc.tile_pool(name="work", bufs=1))

        def bc3(ap, width):
            """[P, k] slice -> [P, k, width] broadcast (stride-0 inner)."""
            return bass.AP(tensor=ap.tensor, offset=ap.offset,
                           ap=[list(ap.ap[0]), list(ap.ap[1]), [0, width]])

        def bc2(ap, width):
            """[P, 1] column -> [P, width] broadcast."""
            return bass.AP(tensor=ap.tensor, offset=ap.offset,
                           ap=[list(ap.ap[0]), [0, width]])

        # ---- constants ----
        ident = singles.tile([128, 128], f32)
        make_identity(nc, ident[:])

        iota_i = singles.tile([128, WIN], i32)
        nc.gpsimd.iota(iota_i[:], pattern=[[1, WIN]], channel_multiplier=0)
        iota_f = singles.tile([128, WIN], f32)
        nc.vector.tensor_copy(iota_f[:], iota_i[:])
        iota_b = bass.AP(tensor=iota_f.tensor, offset=iota_f.offset,
                         ap=[list(iota_f[:].ap[0]), [0, 8], [1, WIN]])

        rmod = singles.tile([16, 128], i32)
        nc.gpsimd.iota(rmod[:].rearrange("p (a b) -> p a b", a=2),
                       pattern=[[0, 2], [320, 64]], channel_multiplier=20)

        ncl_sb = singles.tile([128, 1], i32)
        nc.sync.dma_start(out=ncl_sb[:], in_=ncl_ext[:])
        lab_pg = singles.tile([128, NG], i32)   # [p, g] = labels[g*128+p]
        nc.sync.dma_start(out=lab_pg[:],
                          in_=lab_ext[:].rearrange("(g p) -> p g", p=128))
        lab16 = singles.tile([16, 128], i32)    # [p, s] = labels[s*16+p]
        nc.sync.dma_start(out=lab16[:],
                          in_=lab_ext[:].rearrange("(s p) -> p s", p=16))
        labsl = singles.tile([16, 16], i32)
        nc.sync.dma_start(out=labsl[:], in_=lsl_ext[:])

        # ---- W shard load + transpose -> wT (4 x [128, CPAD]) ----
        wT = [singles.tile([128, CPAD], bf16, tag=f"wT{k}", name=f"wT{k}")
              for k in range(4)]
        for ct in range(CPAD // 128):
            w_t = wt_pool.tile([128, D], f32, tag="wld")
            rows = min(CSH - ct * 128, 128)
            if rows < 128:
                nc.vector.memset(w_t[:], 0.0)
            nc.sync.dma_start(out=w_t[:rows, :],
                              in_=w_ext[ct * 128: ct * 128 + rows, :])
            for k in range(4):
                pt = ps_t.tile([128, 128], f32)
                nc.tensor.transpose(pt[:], w_t[:, k * 128:(k + 1) * 128], ident[:])
                nc.vector.tensor_copy(wT[k][:, ct * 128:(ct + 1) * 128], pt[:])

        # ---- x load: row sq-norms (ACT) + raw transpose (PE) in parallel ----
        xnT = [singles.tile([128, N], bf16, tag=f"xnT{k}", name=f"xnT{k}")
               for k in range(4)]
        ss16 = singles.tile([128, NG], f32)
        for g2 in range(NG // 2):
            xp = xt_pool.tile([128, 2 * D], f32, tag="xA")
            nc.sync.dma_start(
                out=xp[:].rearrange("p (two d) -> p two d", two=2),
                in_=bass.AP(tensor=x_ext, offset=g2 * 256 * D,
                            ap=[[D, 128], [128 * D, 2], [1, D]]))
            for sub in range(2):
                g = 2 * g2 + sub
                xs = xp[:, sub * D:(sub + 1) * D]
                dmp = dump_pool.tile([128, D], f32, tag="sqdump")
                nc.scalar.activation(dmp[:], xs, Act.Square,
                                     accum_out=ss16[:, g:g + 1])
                for k in range(4):
                    pt = ps_t.tile([128, 128], f32)
                    nc.tensor.transpose(pt[:], xp[:, sub * D + k * 128:
                                                  sub * D + (k + 1) * 128],
                                        ident[:])
                    nc.vector.tensor_copy(xnT[k][:, g * 128:(g + 1) * 128], pt[:])
        inv16 = singles.tile([128, NG], f32)
        nc.scalar.activation(inv16[:], ss16[:], Act.Sqrt)
        nc.vector.tensor_scalar_max(inv16[:], inv16[:], 1e-12)
        nc.vector.reciprocal(inv16[:], inv16[:])
        sinv16 = singles.tile([128, NG], f32)
        nc.vector.tensor_scalar_mul(sinv16[:], inv16[:], S_SCALE)

        # ---- label-window bookkeeping (int math) ----
        lpg = singles.tile([128, NG], i32)
        nc.vector.tensor_tensor(lpg[:], lab_pg[:], bc2(ncl_sb[:, 0:1], NG),
                                op=Alu.add)
        bpg = singles.tile([128, NG], i32)
        nc.vector.tensor_scalar(bpg[:], lpg[:], 6, None, op0=Alu.arith_shift_right)
        nc.vector.tensor_scalar(bpg[:], bpg[:], -1, 0, op0=Alu.add, op1=Alu.max)
        nc.vector.tensor_scalar(bpg[:], bpg[:], BMAX, None, op0=Alu.min)
        apg = singles.tile([128, NG], i32)
        nc.vector.tensor_scalar_mul(apg[:], bpg[:], BLK)
        af = singles.tile([128, NG], f32)
        nc.vector.tensor_copy(af[:], apg[:])
        s0i = singles.tile([128, NG], i32)
        nc.vector.tensor_tensor(s0i[:], apg[:], lpg[:], op=Alu.subtract)
        s0f = singles.tile([128, NG], f32)
        nc.vector.tensor_copy(s0f[:], s0i[:])

        l16 = singles.tile([16, 128], i32)
        nc.vector.tensor_tensor(l16[:], lab16[:], bc2(ncl_sb[:16, 0:1], 128),
                                op=Alu.add)
        b16 = singles.tile([16, 128], i32)
        nc.vector.tensor_scalar(b16[:], l16[:], 6, None, op0=Alu.arith_shift_right)
        nc.vector.tensor_scalar(b16[:], b16[:], -1, 0, op0=Alu.add, op1=Alu.max)
        nc.vector.tensor_scalar(b16[:], b16[:], BMAX, None, op0=Alu.min)
        idx32 = singles.tile([16, 128], i32)
        nc.vector.tensor_tensor(idx32[:], b16[:], rmod[:], op=Alu.add)
        idx16 = singles.tile([128, 128], i16)
        nc.vector.memset(idx16[:], 0)
        nc.vector.tensor_copy(idx16[:16, :], idx32[:])

        # ---- normalization Z (analytic geometric sums), [16,16] slice ----
        yf = singles.tile([16, 16], f32)
        nc.vector.tensor_copy(yf[:], labsl[:])
        mL = singles.tile([16, 16], f32)
        nc.vector.tensor_scalar(mL[:], yf[:], float((C // 2) - 1), 1.0,
                                op0=Alu.min, op1=Alu.add)
        gL = singles.tile([16, 16], f32)
        nc.scalar.activation(gL[:], mL[:], Act.Exp, scale=-KDECAY)
        mR = singles.tile([16, 16], f32)
        nc.vector.tensor_scalar(mR[:], yf[:], -1.0, float(C - 1),
                                op0=Alu.mult, op1=Alu.add)
        nc.vector.tensor_scalar(mR[:], mR[:], float(C // 2), 1.0,
                                op0=Alu.min, op1=Alu.add)
        gR = singles.tile([16, 16], f32)
        nc.scalar.activation(gR[:], mR[:], Act.Exp, scale=-KDECAY)
        z_t = singles.tile([16, 16], f32)
        nc.vector.tensor_tensor(z_t[:], gL[:], gR[:], op=Alu.add)
        nc.vector.tensor_scalar(z_t[:], z_t[:], -C2, C1, op0=Alu.mult, op1=Alu.add)
        invz = singles.tile([16, 16], f32)
        nc.vector.reciprocal(invz[:], z_t[:])

        partials = singles.tile([128, 48], f32)
        esum48 = singles.tile([128, NG * 3], f32)

        HALF_ROWS = 1024
        VROWS = (HALF_ROWS - 1) * (CPAD // BLK) + BMAX + 1  # 20478

        def emit_gather(h, eng_win):
            win_t = work.tile([128, 8 * WIN], f32, tag=f"win{h}",
                              name=f"win{h}")
            gth = nc.gpsimd.dma_gather(
                out_ap=win_t[:].rearrange("p (g w) -> p g w", w=WIN),
                in_ap=bass.AP(tensor=(wf0_ext if h == 0 else wf1_ext), offset=0,
                              ap=[[BLK, VROWS], [1, WIN]]),
                idxs_ap=idx16[:, h * 64:(h + 1) * 64],
                num_idxs=HALF_ROWS,
                num_idxs_reg=HALF_ROWS,
                elem_size=WIN,
                elem_step=BLK,
            )
            return win_t, gth

        def win_early(h, eng, win_t, anchor=None):
            """d, |d|, lcol, tcl, tt2 — only needs gathered win + labels."""
            s0_b = bc3(s0f[:, h * 8:(h + 1) * 8], WIN)
            a_b = bc3(af[:, h * 8:(h + 1) * 8], WIN)
            d_t = work.tile([128, 8 * WIN], f32, tag=f"d{h}", name=f"d{h}")
            nc_e = getattr(nc, eng)
            roots = []
            roots.append(nc_e.tensor_tensor(
                d_t[:].rearrange("p (g w) -> p g w", w=WIN),
                iota_b, s0_b, op=Alu.add))
            ad_t = work.tile([128, 8 * WIN], f32, tag=f"ad{h}", name=f"ad{h}")
            nc_e.scalar_tensor_tensor(ad_t[:], d_t[:], -1.0, d_t[:],
                                      op0=Alu.mult, op1=Alu.max)
            lc_t = work.tile([128, 8 * WIN], f32, tag=f"lc{h}", name=f"lc{h}")
            roots.append(nc_e.tensor_tensor(
                lc_t[:].rearrange("p (g w) -> p g w", w=WIN),
                iota_b, a_b, op=Alu.add))
            tcl = work.tile([128, 8 * WIN], f32, tag=f"tcl{h}", name=f"tcl{h}")
            roots.append(nc_e.tensor_scalar(tcl[:], win_t[:], -CLIP, CLIP,
                                            op0=Alu.max, op1=Alu.min))
            tt2 = work.tile([128, 8 * WIN], f32, tag=f"tt2{h}", name=f"tt2{h}")
            nc_e.tensor_tensor(tt2[:], tcl[:], tcl[:], op=Alu.mult)
            if anchor is not None:
                for r in roots:
                    _add_dep_helper(r.ins, anchor.ins, False,
                                    "pin window ops late in stream")
            return d_t, ad_t, lc_t, tcl, tt2

        def win_act(h, ad_t, tt2, anchor=None):
            kern = work.tile([128, 8 * WIN], f32, tag=f"kern{h}", name=f"kern{h}")
            k_i = nc.scalar.activation(kern[:], ad_t[:], Act.Exp, scale=-KDECAY)
            s2_t = work.tile([128, 8 * WIN], f32, tag=f"s2{h}", name=f"s2{h}")
            s_i = nc.scalar.activation(s2_t[:], tt2[:], Act.Sqrt,
                                       scale=-1.0, bias=1.0)
            if anchor is not None:
                _add_dep_helper(k_i.ins, anchor.ins, False, "pin act late")
                _add_dep_helper(s_i.ins, anchor.ins, False, "pin act late")
            return kern, s2_t

        def win_late(h, eng, win_t, d_t, lc_t, tcl, kern, s2_t):
            """mask kern, u, c1; reduces go on DVE."""
            nc_e = getattr(nc, eng)
            nc_e.scalar_tensor_tensor(kern[:], lc_t[:], float(CSH), kern[:],
                                      op0=Alu.is_lt, op1=Alu.mult)
            nc_e.scalar_tensor_tensor(s2_t[:], tcl[:], A_OVER_B, s2_t[:],
                                      op0=Alu.mult, op1=Alu.add)
            nc_e.tensor_tensor(kern[:], kern[:], s2_t[:], op=Alu.mult)
            nc_e.scalar_tensor_tensor(win_t[:], d_t[:], 0.0, win_t[:],
                                      op0=Alu.is_equal, op1=Alu.mult)

        def win_reduce(h, win_t, kern):
            base = 16 + 16 * h
            nc.vector.tensor_reduce(
                partials[:, base:base + 8],
                kern[:].rearrange("p (g w) -> p g w", w=WIN),
                axis=mybir.AxisListType.X, op=Alu.add)
            nc.vector.tensor_reduce(
                partials[:, base + 8:base + 16],
                win_t[:].rearrange("p (g w) -> p g w", w=WIN),
                axis=mybir.AxisListType.X, op=Alu.add)

        # ---- main loop: f32r matmul + scaled copy + exp-sum + writeback ----
        wf_dmas = []
        copy_anchor = {}
        exp_anchor = {}
        for g in range(NG):
            wf_t = wf_pool.tile([128, CPAD], f32, tag="wf")
            for ci in range(3):
                cw, cv, c0 = CW[ci], CVALID[ci], ci * 512
                pm = ps_mm.tile([128, 512], f32, tag="mm")
                for k in range(4):
                    nc.tensor.matmul(
                        pm[:, :cw],
                        lhsT=xnT[k][:, g * 128:(g + 1) * 128],
                        rhs=wT[k][:, c0:c0 + cw],
                        start=(k == 0), stop=(k == 3))
                nc.vector.tensor_scalar_mul(wf_t[:, c0:c0 + cw], pm[:, :cw],
                                            inv16[:, g:g + 1])
                dmp = dump_pool.tile([128, 512], f32, tag="expdump")
                nc.scalar.activation(dmp[:, :cv], pm[:, :cv], Act.Exp,
                                     scale=sinv16[:, g:g + 1],
                                     accum_out=esum48[:, g * 3 + ci:g * 3 + ci + 1])
            copy_anchor[g] = cp_i
            exp_anchor[g] = ex_i
            wf_half = wf0_ext if g < 8 else wf1_ext
            r0 = (g % 8) * 128
            dma = nc.sync.dma_start(out=wf_half[r0:r0 + 128, :], in_=wf_t[:])
            wf_dmas.append(dma)

            if g == 7:
                win0, gth0 = emit_gather(0, "gpsimd")
                for dma in wf_dmas[:8]:
                    _add_dep_helper(gth0.ins, dma.ins, True, "gather0 after wf g0-7")
            if g == 12:
                w0_early = win_early(0, "vector", win0, anchor=copy_anchor[11])
            if g == 13:
                d0, ad0, lc0, tcl0, tt20 = w0_early
                kern0, s20 = win_act(0, ad0, tt20, anchor=exp_anchor[12])
            if g == 14:
                win_late(0, "vector", win0, d0, lc0, tcl0, kern0, s20)

        # RS_a: exp-sums only — trigger queued BEFORE gather1's Q7 launch
        # so the collective flies while both window phases compute
        nc.vector.tensor_reduce(
            partials[:, 0:NG],
            esum48[:].rearrange("p (g c) -> p g c", c=3),
            axis=mybir.AxisListType.X, op=Alu.add)
        nc.sync.dma_start(
            out=ara_in[:].rearrange("(p f) -> p f", p=128),
            in_=partials[:, 0:16])
        nc.gpsimd.collective_compute(
            "ReduceScatter", mybir.AluOpType.add,
            replica_groups=[list(range(NCORES))],
            ins=[ara_in[:]], outs=[rsa_out[:]])
        red_a = singles.tile([16, 16], f32)
        nc.sync.dma_start(out=red_a[:],
                          in_=rsa_out[:].rearrange("(p f) -> p f", p=16))

        win1, gth1 = emit_gather(1, "vector")
        for dma in wf_dmas[8:]:
            _add_dep_helper(gth1.ins, dma.ins, True, "gather1 after wf g8-15")

        win_reduce(0, win0, kern0)
        # ---- second-half window phase (post-loop, DVE has idle time) ----
        d1, ad1, lc1, tcl1, tt21 = win_early(1, "vector", win1)
        kern1, s21 = win_act(1, ad1, tt21)
        win_late(1, "vector", win1, d1, lc1, tcl1, kern1, s21)
        win_reduce(1, win1, kern1)

        # RS_b: all window terms (t1/wfy for both halves)
        nc.sync.dma_start(
            out=arb_in[:].rearrange("(p f) -> p f", p=128),
            in_=partials[:, 16:48])
        nc.gpsimd.collective_compute(
            "ReduceScatter", mybir.AluOpType.add,
            replica_groups=[list(range(NCORES))],
            ins=[arb_in[:]], outs=[rsb_out[:]])
        red_b = singles.tile([16, 32], f32)
        nc.sync.dma_start(out=red_b[:],
                          in_=rsb_out[:].rearrange("(p f) -> p f", p=16))
        esumT = red_a[:, 0:16]
        t1T = singles.tile([16, 16], f32)
        nc.vector.tensor_copy(t1T[:, 0:8], red_b[:, 0:8])
        nc.vector.tensor_copy(t1T[:, 8:16], red_b[:, 16:24])
        wfyT = singles.tile([16, 16], f32)
        nc.vector.tensor_copy(wfyT[:, 0:8], red_b[:, 8:16])
        nc.vector.tensor_copy(wfyT[:, 8:16], red_b[:, 24:32])
        t1T = t1T[:]
        wfyT = wfyT[:]

        # ---- final per-row loss slice ----
        tcy = singles.tile([16, 16], f32)
        nc.vector.tensor_scalar(tcy[:], wfyT, -CLIP, CLIP, op0=Alu.max, op1=Alu.min)
        ty2 = singles.tile([16, 16], f32)
        nc.vector.tensor_tensor(ty2[:], tcy[:], tcy[:], op=Alu.mult)
        s2y = singles.tile([16, 16], f32)
        nc.scalar.activation(s2y[:], ty2[:], Act.Sqrt, scale=-1.0, bias=1.0)
        tyA = singles.tile([16, 16], f32)
        nc.vector.tensor_scalar_mul(tyA[:], tcy[:], A_COS)
        numy = singles.tile([16, 16], f32)
        nc.vector.scalar_tensor_tensor(numy[:], s2y[:], B_SIN, tyA[:],
                                       op0=Alu.mult, op1=Alu.add)
        eny = singles.tile([16, 16], f32)
        nc.scalar.activation(eny[:], numy[:], Act.Exp)
        ey = singles.tile([16, 16], f32)
        nc.scalar.activation(ey[:], wfyT, Act.Exp, scale=S_SCALE)
        den = singles.tile([16, 16], f32)
        nc.vector.tensor_tensor(den[:], eny[:], esumT, op=Alu.add)
        nc.vector.tensor_tensor(den[:], den[:], ey[:], op=Alu.subtract)
        lden = singles.tile([16, 16], f32)
        nc.scalar.activation(lden[:], den[:], Act.Ln)
        q1 = singles.tile([16, 16], f32)
        nc.vector.tensor_tensor(q1[:], t1T, invz[:], op=Alu.mult)
        l_t = singles.tile([16, 16], f32)
        nc.vector.scalar_tensor_tensor(l_t[:], q1[:], B_SIN, lden[:],
                                       op0=Alu.mult, op1=Alu.subtract)
        nc.sync.dma_start(out=l_ext[:].rearrange("(pp g) -> pp g", g=16),
                          in_=l_t[:])

    nc.finalize()
    return nc


def _get_nc():
    if "nc" not in _CACHE:
        _CACHE["nc"] = _build()
    return _CACHE["nc"]


def make_in_maps(x, labels, W):
    x = np.ascontiguousarray(x, dtype=np.float32)
    W = np.ascontiguousarray(W, dtype=np.float32)
    labels = np.ascontiguousarray(labels, dtype=np.int32)
    lab2d = labels.reshape(NG, 128)  # [g, p]
    in_maps = []
    for i in range(NCORES):
        # labslice[pp, g] = labels[g*128 + 16*i + pp]
        lsl = np.ascontiguousarray(lab2d[:, 16 * i:16 * (i + 1)].T)
        in_maps.append({
            "xsl": np.ascontiguousarray(x[i * 256:(i + 1) * 256]),
            "w": np.ascontiguousarray(W[i * CSH:(i + 1) * CSH]),
            "labels": labels,
            "negclo": np.full((128, 1), -i * CSH, dtype=np.int32),
            "labslice": lsl,
        })
    return in_maps


def assemble(outs):
    """outs: per-core dicts with 'wf_out0'/'wf_out1' [N/2, CPAD], 'l_out' [256]."""
    wf = np.concatenate(
        [np.concatenate([outs[i]["wf_out0"], outs[i]["wf_out1"]], axis=0)[:, :CSH]
         for i in range(NCORES)], axis=1)
    l2d = np.zeros((NG, 128), dtype=np.float32)  # [g, p]
    for i in range(NCORES):
        l2d[:, 16 * i:16 * (i + 1)] = outs[i]["l_out"].reshape(16, 16).T
    loss = np.float32(-np.mean(l2d))
    return wf, loss


def kernel(x, labels, W):
    from concourse.bass_utils import run_bass_kernel_spmd

    nc = _get_nc()
    in_maps = make_in_maps(x, labels, W)
    res = run_bass_kernel_spmd(nc, in_maps, core_ids=list(range(NCORES)))
    return assemble(res.results)


# revision 27
# speedup vs baseline: 1.8814x; 1.1165x over previous
"""Trainium2 Bass kernel for AngularPenaltySMLossWithSoftLabel.

Strategy: tensor-parallel over out_features C=10000 across 8 cores
(1250 columns each). Each core:
  - transposes its W shard and the full (raw) x on the PE,
  - computes wf[:, c_lo:c_hi] = x @ W_shard.T in float32r, scaling rows
    by 1/||x_n|| in the epilogue (copy + fused exp-sum accumulate),
  - gathers a 192-wide window around each row's label (clipped to its
    shard) from its wf shard via indirect DMA and computes the
    soft-label kernel * arcface-numerator contribution (the
    exp(-0.2*d) kernel is < 3e-6 outside +-64),
  - ReduceScatters [exp_sum, term1, wf_y] (3 x 2048 f32) across cores,
  - computes the final per-row loss slice L (256 rows/core) on device.
Host glue: concatenate wf shards, slice padding, loss = -mean(L).
"""

import math
import numpy as np

N, D, C = 2048, 512, 10000
NCORES = 8
CSH = C // NCORES          # 1250 columns per core
CPAD = 1280                # padded row stride of the wf output (256B multiple)
S_SCALE = 64.0
MARGIN = 0.5
EPS = 1e-7
KDECAY = 0.2

A_COS = S_SCALE * math.cos(MARGIN)            # 56.1652...
B_SIN = -S_SCALE * math.sin(MARGIN)           # -30.6832...
A_OVER_B = math.cos(MARGIN) / -math.sin(MARGIN)  # -1.8304877...
CLIP = 1.0 - EPS
G = math.exp(-KDECAY)
C2 = 1.0 / (1.0 - G)                          # 5.51667
C1 = 1.0 + 2.0 * G * C2                       # 10.03331

WIN = 192          # gather window width (64-aligned, covers label +-64)
BLK = 64           # gather stride granularity (256B in f32)
BMAX = (CPAD - WIN) // BLK   # 17: max window block start
NG = N // 128      # 16 row-chunks of 128
CW = [512, 512, 256]         # c-chunk widths (local columns)
CVALID = [512, 512, CSH - 1024]  # valid (non-ghost) widths

_CACHE = {}


def _build():
    import concourse.bass as bass
    import concourse.mybir as mybir
    from concourse import bacc, tile
    from concourse.masks import make_identity
    from concourse.bass import _add_dep_helper
    from contextlib import ExitStack

    f32 = mybir.dt.float32
    f32r = mybir.dt.float32r
    bf16 = mybir.dt.bfloat16
    i32 = mybir.dt.int32
    i16 = mybir.dt.int16
    Alu = mybir.AluOpType
    Act = mybir.ActivationFunctionType

    nc = bacc.Bacc("TRN2", num_devices=NCORES, num_swdge_queues=2)

    x_ext = nc.declare_dram_parameter("x", [N, D], f32, isOutput=False)
    w_ext = nc.declare_dram_parameter("w", [CSH, D], f32, isOutput=False)
    lab_ext = nc.declare_dram_parameter("labels", [N], i32, isOutput=False)
    ncl_ext = nc.declare_dram_parameter("negclo", [128, 1], i32, isOutput=False)
    lsl_ext = nc.declare_dram_parameter("labslice", [16, 16], i32, isOutput=False)
    wf0_ext = nc.declare_dram_parameter("wf_out0", [N // 2, CPAD], f32,
                                        isOutput=True)
    wf1_ext = nc.declare_dram_parameter("wf_out1", [N // 2, CPAD], f32,
                                        isOutput=True)
    l_ext = nc.declare_dram_parameter("l_out", [256], f32, isOutput=True)

    ar_in = nc.dram_tensor("ar_in", [128 * 48], f32)
    rs_out = nc.dram_tensor("rs_out", [16 * 48], f32)

    with ExitStack() as ctx:
        tc = ctx.enter_context(tile.TileContext(nc))
        singles = ctx.enter_context(tc.tile_pool(name="singles", bufs=1))
        xt_pool = ctx.enter_context(tc.tile_pool(name="xt", bufs=3))
        wt_pool = ctx.enter_context(tc.tile_pool(name="wt", bufs=3))
        ps_t = ctx.enter_context(tc.tile_pool(name="pst", bufs=2, space="PSUM"))
        ps_mm = ctx.enter_context(tc.tile_pool(name="psmm", bufs=6, space="PSUM"))
        wf_pool = ctx.enter_context(tc.tile_pool(name="wfout", bufs=6))
        dump_pool = ctx.enter_context(tc.tile_pool(name="edump", bufs=2))
        work = ctx.enter_context(tc.tile_pool(name="work", bufs=1))

        def bc3(ap, width):
            """[P, k] slice -> [P, k, width] broadcast (stride-0 inner)."""
            return bass.AP(tensor=ap.tensor, offset=ap.offset,
                           ap=[list(ap.ap[0]), list(ap.ap[1]), [0, width]])

        def bc2(ap, width):
            """[P, 1] column -> [P, width] broadcast."""
            return bass.AP(tensor=ap.tensor, offset=ap.offset,
                           ap=[list(ap.ap[0]), [0, width]])

        # ---- constants ----
        ident = singles.tile([128, 128], f32)
        make_identity(nc, ident[:])

        iota_i = singles.tile([128, WIN], i32)
        nc.gpsimd.iota(iota_i[:], pattern=[[1, WIN]], channel_multiplier=0)
        iota_f = singles.tile([128, WIN], f32)
        nc.vector.tensor_copy(iota_f[:], iota_i[:])
        iota_b = bass.AP(tensor=iota_f.tensor, offset=iota_f.offset,
                         ap=[list(iota_f[:].ap[0]), [0, 8], [1, WIN]])

        rmod = singles.tile([16, 128], i32)
        nc.gpsimd.iota(rmod[:].rearrange("p (a b) -> p a b", a=4),
                       pattern=[[0, 4], [320, 32]], channel_multiplier=20)

        ncl_sb = singles.tile([128, 1], i32)
        nc.sync.dma_start(out=ncl_sb[:], in_=ncl_ext[:])
        lab_pg = singles.tile([128, NG], i32)   # [p, g] = labels[g*128+p]
        nc.sync.dma_start(out=lab_pg[:],
                          in_=lab_ext[:].rearrange("(g p) -> p g", p=128))
        lab16 = singles.tile([16, 128], i32)    # [p, s] = labels[s*16+p]
        nc.sync.dma_start(out=lab16[:],
                          in_=lab_ext[:].rearrange("(s p) -> p s", p=16))
        labsl = singles.tile([16, 16], i32)
        nc.sync.dma_start(out=labsl[:], in_=lsl_ext[:])

        # ---- W shard load + transpose -> wT (4 x [128, CPAD]) ----
        wT = [singles.tile([128, CPAD], bf16, tag=f"wT{k}", name=f"wT{k}")
              for k in range(4)]
        for ct in range(CPAD // 128):
            w_t = wt_pool.tile([128, D], f32, tag="wld")
            rows = min(CSH - ct * 128, 128)
            if rows < 128:
                nc.vector.memset(w_t[:], 0.0)
            nc.sync.dma_start(out=w_t[:rows, :],
                              in_=w_ext[ct * 128: ct * 128 + rows, :])
            for k in range(4):
                pt = ps_t.tile([128, 128], f32)
                nc.tensor.transpose(pt[:], w_t[:, k * 128:(k + 1) * 128], ident[:])
                nc.vector.tensor_copy(wT[k][:, ct * 128:(ct + 1) * 128], pt[:])

        # ---- x load: row sq-norms (ACT) + raw transpose (PE) in parallel ----
        xnT = [singles.tile([128, N], bf16, tag=f"xnT{k}", name=f"xnT{k}")
               for k in range(4)]
        ss16 = singles.tile([128, NG], f32)
        for g2 in range(NG // 2):
            xp = xt_pool.tile([128, 2 * D], f32, tag="xA")
            nc.sync.dma_start(
                out=xp[:].rearrange("p (two d) -> p two d", two=2),
                in_=bass.AP(tensor=x_ext, offset=g2 * 256 * D,
                            ap=[[D, 128], [128 * D, 2], [1, D]]))
            for sub in range(2):
                g = 2 * g2 + sub
                xs = xp[:, sub * D:(sub + 1) * D]
                dmp = dump_pool.tile([128, D], f32, tag="sqdump")
                nc.scalar.activation(dmp[:], xs, Act.Square,
                                     accum_out=ss16[:, g:g + 1])
                for k in range(4):
                    pt = ps_t.tile([128, 128], f32)
                    nc.tensor.transpose(pt[:], xp[:, sub * D + k * 128:
                                                  sub * D + (k + 1) * 128],
                                        ident[:])
                    nc.vector.tensor_copy(xnT[k][:, g * 128:(g + 1) * 128], pt[:])
        inv16 = singles.tile([128, NG], f32)
        nc.scalar.activation(inv16[:], ss16[:], Act.Sqrt)
        nc.vector.tensor_scalar_max(inv16[:], inv16[:], 1e-12)
        nc.vector.reciprocal(inv16[:], inv16[:])
        sinv16 = singles.tile([128, NG], f32)
        nc.vector.tensor_scalar_mul(sinv16[:], inv16[:], S_SCALE)

        # ---- label-window bookkeeping (int math) ----
        lpg = singles.tile([128, NG], i32)
        nc.vector.tensor_tensor(lpg[:], lab_pg[:], bc2(ncl_sb[:, 0:1], NG),
                                op=Alu.add)
        bpg = singles.tile([128, NG], i32)
        nc.vector.tensor_scalar(bpg[:], lpg[:], 6, None, op0=Alu.arith_shift_right)
        nc.vector.tensor_scalar(bpg[:], bpg[:], -1, 0, op0=Alu.add, op1=Alu.max)
        nc.vector.tensor_scalar(bpg[:], bpg[:], BMAX, None, op0=Alu.min)
        apg = singles.tile([128, NG], i32)
        nc.vector.tensor_scalar_mul(apg[:], bpg[:], BLK)
        af = singles.tile([128, NG], f32)
        nc.vector.tensor_copy(af[:], apg[:])
        s0i = singles.tile([128, NG], i32)
        nc.vector.tensor_tensor(s0i[:], apg[:], lpg[:], op=Alu.subtract)
        s0f = singles.tile([128, NG], f32)
        nc.vector.tensor_copy(s0f[:], s0i[:])

        l16 = singles.tile([16, 128], i32)
        nc.vector.tensor_tensor(l16[:], lab16[:], bc2(ncl_sb[:16, 0:1], 128),
                                op=Alu.add)
        b16 = singles.tile([16, 128], i32)
        nc.vector.tensor_scalar(b16[:], l16[:], 6, None, op0=Alu.arith_shift_right)
        nc.vector.tensor_scalar(b16[:], b16[:], -1, 0, op0=Alu.add, op1=Alu.max)
        nc.vector.tensor_scalar(b16[:], b16[:], BMAX, None, op0=Alu.min)
        idx32 = singles.tile([16, 128], i32)
        nc.vector.tensor_tensor(idx32[:], b16[:], rmod[:], op=Alu.add)
        idx16 = singles.tile([128, 128], i16)
        nc.vector.memset(idx16[:], 0)
        nc.vector.tensor_copy(idx16[:16, :], idx32[:])

        # ---- normalization Z (analytic geometric sums), [16,16] slice ----
        yf = singles.tile([16, 16], f32)
        nc.vector.tensor_copy(yf[:], labsl[:])
        mL = singles.tile([16, 16], f32)
        nc.vector.tensor_scalar(mL[:], yf[:], float((C // 2) - 1), 1.0,
                                op0=Alu.min, op1=Alu.add)
        gL = singles.tile([16, 16], f32)
        nc.scalar.activation(gL[:], mL[:], Act.Exp, scale=-KDECAY)
        mR = singles.tile([16, 16], f32)
        nc.vector.tensor_scalar(mR[:], yf[:], -1.0, float(C - 1),
                                op0=Alu.mult, op1=Alu.add)
        nc.vector.tensor_scalar(mR[:], mR[:], float(C // 2), 1.0,
                                op0=Alu.min, op1=Alu.add)
        gR = singles.tile([16, 16], f32)
        nc.scalar.activation(gR[:], mR[:], Act.Exp, scale=-KDECAY)
        z_t = singles.tile([16, 16], f32)
        nc.vector.tensor_tensor(z_t[:], gL[:], gR[:], op=Alu.add)
        nc.vector.tensor_scalar(z_t[:], z_t[:], -C2, C1, op0=Alu.mult, op1=Alu.add)
        invz = singles.tile([16, 16], f32)
        nc.vector.reciprocal(invz[:], z_t[:])

        partials = singles.tile([128, 48], f32)
        esum48 = singles.tile([128, NG * 3], f32)

        QROWS = 512
        VROWS = (QROWS - 1) * (CPAD // BLK) + BMAX + 1

        def emit_gather_q(h, q, win_t):
            """512-row quarter gather: rows [1024h+512q, +512) on queue q."""
            gth = nc.gpsimd.dma_gather(
                out_ap=win_t[:].rearrange("p (g w) -> p g w", w=WIN)[:, 4*q:4*(q+1), :],
                in_ap=bass.AP(tensor=(wf0_ext if h == 0 else wf1_ext),
                              offset=512 * q * CPAD,
                              ap=[[BLK, VROWS], [1, WIN]]),
                idxs_ap=idx16[:, h * 64 + q * 32: h * 64 + (q + 1) * 32],
                num_idxs=512,
                num_idxs_reg=512,
                elem_size=WIN,
                elem_step=BLK,
                queue_num=q,
            )
            return gth

        def win_early(h, eng, win_t, anchor=None):
            """d, |d|, lcol, tcl, tt2 — only needs gathered win + labels."""
            s0_b = bc3(s0f[:, h * 8:(h + 1) * 8], WIN)
            a_b = bc3(af[:, h * 8:(h + 1) * 8], WIN)
            d_t = work.tile([128, 8 * WIN], f32, tag=f"d{h}", name=f"d{h}")
            nc_e = getattr(nc, eng)
            roots = []
            roots.append(nc_e.tensor_tensor(
                d_t[:].rearrange("p (g w) -> p g w", w=WIN),
                iota_b, s0_b, op=Alu.add))
            ad_t = work.tile([128, 8 * WIN], f32, tag=f"ad{h}", name=f"ad{h}")
            nc_e.scalar_tensor_tensor(ad_t[:], d_t[:], -1.0, d_t[:],
                                      op0=Alu.mult, op1=Alu.max)
            lc_t = work.tile([128, 8 * WIN], f32, tag=f"lc{h}", name=f"lc{h}")
            roots.append(nc_e.tensor_tensor(
                lc_t[:].rearrange("p (g w) -> p g w", w=WIN),
                iota_b, a_b, op=Alu.add))
            tcl = work.tile([128, 8 * WIN], f32, tag=f"tcl{h}", name=f"tcl{h}")
            roots.append(nc_e.tensor_scalar(tcl[:], win_t[:], -CLIP, CLIP,
                                            op0=Alu.max, op1=Alu.min))
            tt2 = work.tile([128, 8 * WIN], f32, tag=f"tt2{h}", name=f"tt2{h}")
            nc_e.tensor_tensor(tt2[:], tcl[:], tcl[:], op=Alu.mult)
            if anchor is not None:
                for r in roots:
                    _add_dep_helper(r.ins, anchor.ins, False,
                                    "pin window ops late in stream")
            return d_t, ad_t, lc_t, tcl, tt2

        def win_act(h, ad_t, tt2, anchor=None):
            kern = work.tile([128, 8 * WIN], f32, tag=f"kern{h}", name=f"kern{h}")
            k_i = nc.scalar.activation(kern[:], ad_t[:], Act.Exp, scale=-KDECAY)
            s2_t = work.tile([128, 8 * WIN], f32, tag=f"s2{h}", name=f"s2{h}")
            s_i = nc.scalar.activation(s2_t[:], tt2[:], Act.Sqrt,
                                       scale=-1.0, bias=1.0)
            if anchor is not None:
                _add_dep_helper(k_i.ins, anchor.ins, False, "pin act late")
                _add_dep_helper(s_i.ins, anchor.ins, False, "pin act late")
            return kern, s2_t

        def win_late(h, eng, win_t, d_t, lc_t, tcl, kern, s2_t):
            """mask kern, u, c1; reduces go on DVE."""
            nc_e = getattr(nc, eng)
            nc_e.scalar_tensor_tensor(kern[:], lc_t[:], float(CSH), kern[:],
                                      op0=Alu.is_lt, op1=Alu.mult)
            nc_e.scalar_tensor_tensor(s2_t[:], tcl[:], A_OVER_B, s2_t[:],
                                      op0=Alu.mult, op1=Alu.add)
            nc_e.tensor_tensor(kern[:], kern[:], s2_t[:], op=Alu.mult)
            nc_e.scalar_tensor_tensor(win_t[:], d_t[:], 0.0, win_t[:],
                                      op0=Alu.is_equal, op1=Alu.mult)

        def win_reduce(h, win_t, kern):
            base = 16 + 16 * h
            nc.vector.tensor_reduce(
                partials[:, base:base + 8],
                kern[:].rearrange("p (g w) -> p g w", w=WIN),
                axis=mybir.AxisListType.X, op=Alu.add)
            nc.vector.tensor_reduce(
                partials[:, base + 8:base + 16],
                win_t[:].rearrange("p (g w) -> p g w", w=WIN),
                axis=mybir.AxisListType.X, op=Alu.add)

        # ---- main loop: f32r matmul + scaled copy + exp-sum + writeback ----
        wf_dmas = []
        copy_anchor = {}
        exp_anchor = {}
        for g in range(NG):
            wf_t = wf_pool.tile([128, CPAD], f32, tag="wf")
            for ci in range(3):
                cw, cv, c0 = CW[ci], CVALID[ci], ci * 512
                pm = ps_mm.tile([128, 512], f32, tag="mm")
                for k in range(4):
                    nc.tensor.matmul(
                        pm[:, :cw],
                        lhsT=xnT[k][:, g * 128:(g + 1) * 128],
                        rhs=wT[k][:, c0:c0 + cw],
                        start=(k == 0), stop=(k == 3))
                nc.vector.tensor_scalar_mul(wf_t[:, c0:c0 + cw], pm[:, :cw],
                                            inv16[:, g:g + 1])
                dmp = dump_pool.tile([128, 512], f32, tag="expdump")
                nc.scalar.activation(dmp[:, :cv], pm[:, :cv], Act.Exp,
                                     scale=sinv16[:, g:g + 1],
                                     accum_out=esum48[:, g * 3 + ci:g * 3 + ci + 1])
            copy_anchor[g] = cp_i
            exp_anchor[g] = ex_i
            wf_half = wf0_ext if g < 8 else wf1_ext
            r0 = (g % 8) * 128
            dma = nc.sync.dma_start(out=wf_half[r0:r0 + 128, :], in_=wf_t[:])
            wf_dmas.append(dma)

            if g == 3:
                win0 = work.tile([128, 8 * WIN], f32, tag="win0", name="win0")
                gq = emit_gather_q(0, 0, win0)
                for dma in wf_dmas[:4]:
                    _add_dep_helper(gq.ins, dma.ins, True, "gather0q0 deps")
            if g == 7:
                gq = emit_gather_q(0, 1, win0)
                for dma in wf_dmas[4:8]:
                    _add_dep_helper(gq.ins, dma.ins, True, "gather0q1 deps")
            if g == 11:
                win1 = work.tile([128, 8 * WIN], f32, tag="win1", name="win1")
                gq = emit_gather_q(1, 0, win1)
                for dma in wf_dmas[8:12]:
                    _add_dep_helper(gq.ins, dma.ins, True, "gather1q0 deps")
            if g == 12:
                w0_early = win_early(0, "vector", win0, anchor=copy_anchor[11])
            if g == 13:
                d0, ad0, lc0, tcl0, tt20 = w0_early
                kern0, s20 = win_act(0, ad0, tt20, anchor=exp_anchor[12])
            if g == 14:
                win_late(0, "vector", win0, d0, lc0, tcl0, kern0, s20)

        # RS_a: exp-sums only — trigger queued BEFORE gather1's Q7 launch
        # so the collective flies while both window phases compute
        nc.vector.tensor_reduce(
            partials[:, 0:NG],
            esum48[:].rearrange("p (g c) -> p g c", c=3),
            axis=mybir.AxisListType.X, op=Alu.add)
        nc.sync.dma_start(
            out=ara_in[:].rearrange("(p f) -> p f", p=128),
            in_=partials[:, 0:16])
        nc.gpsimd.collective_compute(
            "ReduceScatter", mybir.AluOpType.add,
            replica_groups=[list(range(NCORES))],
            ins=[ara_in[:]], outs=[rsa_out[:]])
        red_a = singles.tile([16, 16], f32)
        nc.sync.dma_start(out=red_a[:],
                          in_=rsa_out[:].rearrange("(p f) -> p f", p=16))

        gq = emit_gather_q(1, 1, win1)
        for dma in wf_dmas[12:]:
            _add_dep_helper(gq.ins, dma.ins, True, "gather1q1 deps")

        win_reduce(0, win0, kern0)
        # ---- second-half window phase (post-loop, DVE has idle time) ----
        d1, ad1, lc1, tcl1, tt21 = win_early(1, "vector", win1)
        kern1, s21 = win_act(1, ad1, tt21)
        win_late(1, "vector", win1, d1, lc1, tcl1, kern1, s21)
        win_reduce(1, win1, kern1)

        # RS_b: all window terms (t1/wfy for both halves)
        nc.sync.dma_start(
            out=arb_in[:].rearrange("(p f) -> p f", p=128),
            in_=partials[:, 16:48])
        nc.gpsimd.collective_compute(
            "ReduceScatter", mybir.AluOpType.add,
            replica_groups=[list(range(NCORES))],
            ins=[arb_in[:]], outs=[rsb_out[:]])
        red_b = singles.tile([16, 32], f32)
        nc.sync.dma_start(out=red_b[:],
                          in_=rsb_out[:].rearrange("(p f) -> p f", p=16))
        esumT = red_a[:, 0:16]
        t1T = singles.tile([16, 16], f32)
        nc.vector.tensor_copy(t1T[:, 0:8], red_b[:, 0:8])
        nc.vector.tensor_copy(t1T[:, 8:16], red_b[:, 16:24])
        wfyT = singles.tile([16, 16], f32)
        nc.vector.tensor_copy(wfyT[:, 0:8], red_b[:, 8:16])
        nc.vector.tensor_copy(wfyT[:, 8:16], red_b[:, 24:32])
        t1T = t1T[:]
        wfyT = wfyT[:]

        # ---- final per-row loss slice ----
        tcy = singles.tile([16, 16], f32)
        nc.vector.tensor_scalar(tcy[:], wfyT, -CLIP, CLIP, op0=Alu.max, op1=Alu.min)
        ty2 = singles.tile([16, 16], f32)
        nc.vector.tensor_tensor(ty2[:], tcy[:], tcy[:], op=Alu.mult)
        s2y = singles.tile([16, 16], f32)
        nc.scalar.activation(s2y[:], ty2[:], Act.Sqrt, scale=-1.0, bias=1.0)
        tyA = singles.tile([16, 16], f32)
        nc.vector.tensor_scalar_mul(tyA[:], tcy[:], A_COS)
        numy = singles.tile([16, 16], f32)
        nc.vector.scalar_tensor_tensor(numy[:], s2y[:], B_SIN, tyA[:],
                                       op0=Alu.mult, op1=Alu.add)
        eny = singles.tile([16, 16], f32)
        nc.scalar.activation(eny[:], numy[:], Act.Exp)
        ey = singles.tile([16, 16], f32)
        nc.scalar.activation(ey[:], wfyT, Act.Exp, scale=S_SCALE)
        den = singles.tile([16, 16], f32)
        nc.vector.tensor_tensor(den[:], eny[:], esumT, op=Alu.add)
        nc.vector.tensor_tensor(den[:], den[:], ey[:], op=Alu.subtract)
        lden = singles.tile([16, 16], f32)
        nc.scalar.activation(lden[:], den[:], Act.Ln)
        q1 = singles.tile([16, 16], f32)
        nc.vector.tensor_tensor(q1[:], t1T, invz[:], op=Alu.mult)
        l_t = singles.tile([16, 16], f32)
        nc.vector.scalar_tensor_tensor(l_t[:], q1[:], B_SIN, lden[:],
                                       op0=Alu.mult, op1=Alu.subtract)
        nc.sync.dma_start(out=l_ext[:].rearrange("(pp g) -> pp g", g=16),
                          in_=l_t[:])

    nc.finalize()
    return nc


def _get_nc():
    if "nc" not in _CACHE:
        _CACHE["nc"] = _build()
    return _CACHE["nc"]


def make_in_maps(x, labels, W):
    x = np.ascontiguousarray(x, dtype=np.float32)
    W = np.ascontiguousarray(W, dtype=np.float32)
    labels = np.ascontiguousarray(labels, dtype=np.int32)
    lab2d = labels.reshape(NG, 128)  # [g, p]
    in_maps = []
    for i in range(NCORES):
        # labslice[pp, g] = labels[g*128 + 16*i + pp]
        lsl = np.ascontiguousarray(lab2d[:, 16 * i:16 * (i + 1)].T)
        in_maps.append({
            "xsl": np.ascontiguousarray(x[i * 256:(i + 1) * 256]),
            "w": np.ascontiguousarray(W[i * CSH:(i + 1) * CSH]),
            "labels": labels,
            "negclo": np.full((128, 1), -i * CSH, dtype=np.int32),
            "labslice": lsl,
        })
    return in_maps


def assemble(outs):
    """outs: per-core dicts with 'wf_out0'/'wf_out1' [N/2, CPAD], 'l_out' [256]."""
    wf = np.concatenate(
        [np.concatenate([outs[i]["wf_out0"], outs[i]["wf_out1"]], axis=0)[:, :CSH]
         for i in range(NCORES)], axis=1)
    l2d = np.zeros((NG, 128), dtype=np.float32)  # [g, p]
    for i in range(NCORES):
        l2d[:, 16 * i:16 * (i + 1)] = outs[i]["l_out"].reshape(16, 16).T
    loss = np.float32(-np.mean(l2d))
    return wf, loss


def kernel(x, labels, W):
    from concourse.bass_utils import run_bass_kernel_spmd

    nc = _get_nc()
    in_maps = make_in_maps(x, labels, W)
    res = run_bass_kernel_spmd(nc, in_maps, core_ids=list(range(NCORES)))
    return assemble(res.results)
